# revision 1
# baseline (speedup 1.0000x reference)
"""GAT (3 convs) + Set2Set + MLP on 8 Trainium2 NeuronCores.

Sharding: nodes in 8 equal ranges of 6250; edges (incl self-loops) sorted by
dst and sharded by dst range so the per-dst segment softmax is core-local.
Per conv every core computes the full xl = h @ W_ext (attention dots fused as
extra columns), writes node-major rows to HBM; the edge phase dma_gathers
xl rows by src, builds per-128-edge one-hot*exp matrices and uses PE matmuls
to accumulate messages + softmax denominators per 128-dst-node window.
h is all-gathered between convs. Set2Set+MLP run per-core on a 16-graph slice.
"""
import os
import sys
import types

import numpy as np

sys.path.insert(0, "/opt/trn_rl_repo")

N, E, F_RAW, D, H, B = 50000, 800000, 9, 128, 2, 128
NUM_CONVS = int(os.environ.get("K_CONVS", "3"))
AGGR_STEPS = int(os.environ.get("K_STEPS", "3"))
NEG_SLOPE = 0.2
NCORES = 8
SHARD = N // NCORES            # 6250
HALF = N // 2                  # 25000
NW = (SHARD + 127) // 128      # 49 windows per core
LASTW = SHARD - (NW - 1) * 128 # 106
ROWF = 320                     # padded xl_ext row (f32) -> 1280B, %256 ok
ROWU = 260                     # used cols: xl0(128) 1 xl1(128) 1 asrc(2)

_cached = {}


# ---------------------------------------------------------------- patches
def _install_patches():
    import concourse.tile as tile_mod
    from concourse.vector_clock import ScopedClock, VectorClock

    if not getattr(tile_mod.TileContext, "_drain_patched", False):
        def patched(self, tick_clock, wait_clock):
            gc = tick_clock.global_clock
            vals = [gc[p] for p in range(27)]
            for p in [p for p in range(27) if vals[p] > 0]:
                sub = [vals[q] if q == p else 0 for q in range(27)]
                nop = self.nc.sync.nop(nofuse=True, hint="drain_wait_split")
                wait_clock.add_sem_waits(
                    nop.ins, ScopedClock({None: VectorClock(sub)}))
            self.nc.sync.drain()
            self.nc.all_engine_barrier()
            popped = self.nc._tile_sem_poison_stack.pop()
            assert popped is self._sem_poison
            self.nc.clear_and_free_semaphores(
                list(self.sems.allocated().values()))
            self.nc.all_engine_barrier()

        tile_mod.TileContext._drain_and_barrier = patched
        tile_mod.TileContext._drain_patched = True


def _split_waits(nc, max_waits=1):
    """walrus here allows at most one sync-wait command per instruction;
    spread extras across injected same-engine NoOps."""
    from concourse import mybir
    n = 0
    for f in nc.m.functions:
        for bb in f.blocks:
            changed, new = False, []
            for ins in bb.instructions:
                si = ins.sync_info
                if si is not None and len(si.on_wait) > max_waits:
                    waits = list(si.on_wait)
                    for i, w in enumerate(waits[max_waits:]):
                        nop = mybir.InstNoOp(
                            name=f"{ins.name}-ws{i}", ins=[], outs=[])
                        nop.engine = ins.engine
                        nop.sync_info = mybir.SyncInfo(
                            on_wait=[w], on_update=[])
                        new.append(nop)
                    ins.sync_info = mybir.SyncInfo(
                        on_wait=waits[:max_waits],
                        on_update=list(si.on_update))
                    changed = True
                    n += 1
                new.append(ins)
            if changed:
                bb.instructions = new
    return n


# ---------------------------------------------------------------- host prep
def _wrap16(flat):
    """dma_gather index layout: idx k at [k%16, k//16], replicated to 128."""
    k = flat.shape[0]
    w = flat.reshape(k // 16, 16).T.astype(np.int16)
    return np.tile(w, (8, 1))


def _host_prep(x, edge_index, batch_index, gat_W, gat_att_src, gat_att_dst):
    cfg = {}
    src = np.concatenate([edge_index[0], np.arange(N, dtype=np.int64)])
    dst = np.concatenate([edge_index[1], np.arange(N, dtype=np.int64)])
    order = np.argsort(dst, kind="stable")
    src, dst = src[order].astype(np.int32), dst[order].astype(np.int32)

    # per (core, window, half): edge lists
    EP = len(src)
    core_of = dst // SHARD
    win_of = (dst % SHARD) // 128
    half_of = (src >= HALF).astype(np.int32)
    key = ((core_of * NW + win_of) * 2 + half_of)
    korder = np.argsort(key, kind="stable")
    src_s, dst_s, key_s = src[korder], dst[korder], key[korder]
    counts = np.bincount(key_s, minlength=NCORES * NW * 2).reshape(
        NCORES, NW, 2)
    SA = int(np.ceil(counts[:, :, 0].max() / 128))
    SB = int(np.ceil(counts[:, :, 1].max() / 128))
    SW = SA + SB
    cfg["SA"], cfg["SB"], cfg["SW"] = SA, SB, SW

    starts = np.zeros(NCORES * NW * 2 + 1, np.int64)
    np.cumsum(np.bincount(key_s, minlength=NCORES * NW * 2), out=starts[1:])

    # graph boundaries for set2set
    goff = np.searchsorted(batch_index, np.arange(B + 1))
    rows_per_core = np.array(
        [goff[16 * (c + 1)] - goff[16 * c] for c in range(NCORES)])
    T = int(np.ceil(rows_per_core.max() / 128))
    cfg["T"] = T

    # replicated weights
    W = np.asarray(gat_W, np.float32)              # [128, 256]
    asrc_v = np.asarray(gat_att_src, np.float32)   # [2, 128]
    adst_v = np.asarray(gat_att_dst, np.float32)
    w_as = np.stack([W[:, h * D:(h + 1) * D] @ asrc_v[h] for h in range(H)],
                    axis=1)                        # [128, 2]
    w_ad = np.stack([W[:, h * D:(h + 1) * D] @ adst_v[h] for h in range(H)],
                    axis=1)
    W_eff = np.zeros((D, 262), np.float32)
    W_eff[:, 0:128] = W[:, 0:128]
    W_eff[:, 129:257] = W[:, 128:256]
    W_eff[:, 258:260] = w_as
    W_eff[:, 260:262] = w_ad
    cfg["W_eff"] = W_eff

    xp = np.zeros((N, D), np.float32)
    xp[:, :F_RAW] = x
    h0T = np.zeros((NCORES * 128, SHARD), np.float32)
    for s in range(NCORES):
        h0T[128 * s:128 * (s + 1), :] = xp[SHARD * s:SHARD * (s + 1)].T
    cfg["h0T"] = h0T

    per_core = []
    ncols_idx = (SA * 128) // 16
    ncols_idx_b = (SB * 128) // 16
    for c in range(NCORES):
        A_idx = np.zeros((NW, 128, ncols_idx), np.int16)
        B_idx = np.zeros((NW, 128, ncols_idx_b), np.int16)
        dstloc = np.full((NW, 128, SW), -1.0, np.float32)
        dstrow = np.full((NW, SW * 128), -1.0, np.float32)
        for w in range(NW):
            for hf, (idx_arr, S_h, ncol) in enumerate(
                    ((A_idx, SA, ncols_idx), (B_idx, SB, ncols_idx_b))):
                k = (c * NW + w) * 2 + hf
                lo, hi = starts[k], starts[k + 1]
                cnt = hi - lo
                flat = np.zeros(S_h * 128, np.int64)  # pad -> row 0 (finite)
                flat[:cnt] = src_s[lo:hi] - HALF * hf
                idx_arr[w] = _wrap16(flat)
                dl = np.full(S_h * 128, -1.0, np.float32)
                dl[:cnt] = (dst_s[lo:hi] % SHARD - 128 * w).astype(np.float32)
                base_slot = 0 if hf == 0 else SA
                for s_ in range(S_h):
                    dstloc[w, :, base_slot + s_] = dl[s_ * 128:(s_ + 1) * 128]
                    dstrow[w, (base_slot + s_) * 128:(base_slot + s_ + 1) * 128] = \
                        dl[s_ * 128:(s_ + 1) * 128]
        adst_widx = np.zeros((NW, 128, 1), np.int32)
        for w in range(NW):
            rows = SHARD * c + 128 * w + np.arange(128)
            adst_widx[w, :, 0] = np.minimum(rows, N - 1)

        # set2set slice
        r0, r1 = goff[16 * c], goff[16 * (c + 1)]
        xidx = np.zeros((T, 128, 1), np.int32)
        bloc = np.full((T, 128, 1), -1.0, np.float32)
        rows = np.arange(T * 128)
        glob = np.minimum(r0 + rows, N - 1)
        xidx[:, :, 0] = glob.reshape(T, 128)
        valid = (r0 + rows) < r1
        bl = np.full(T * 128, -1.0, np.float32)
        bl[valid] = (batch_index[glob[valid]] - 16 * c).astype(np.float32)
        bloc[:, :, 0] = bl.reshape(T, 128)
        brep = np.tile(bl.reshape(T, 1, 128), (1, 16, 1)).astype(np.float32)

        per_core.append(dict(
            A_idx=A_idx, B_idx=B_idx, dstloc=dstloc, dstrow=dstrow,
            adst_widx=adst_widx, s2s_xidx=xidx, s2s_bloc=bloc, s2s_brep=brep,
        ))
    return cfg, per_core


# ---------------------------------------------------------------- device build
def _build(cfg):
    import concourse.bacc as bacc
    import concourse.bass as bass
    import concourse.tile as tile
    from concourse import mybir
    from concourse.masks import make_identity

    _install_patches()
    f32 = mybir.dt.float32
    AF = mybir.ActivationFunctionType
    OP = mybir.AluOpType
    SA, SB, SW, T = cfg["SA"], cfg["SB"], cfg["SW"], cfg["T"]

    DEBUG = bool(int(os.environ.get("K_DEBUG", "0")))
    nc = bacc.Bacc("TRN2", num_swdge_queues=4)
    P_ = nc.declare_dram_parameter
    h0T = P_("h0T", [NCORES * 128, SHARD], f32, isOutput=False)
    W_eff = P_("W_eff", [D, 262], f32, isOutput=False)
    bias_rep = P_("bias_rep", [128, 128], f32, isOutput=False)
    A_idx = P_("A_idx", [NW, 128, SA * 8], mybir.dt.int16, isOutput=False)
    B_idx = P_("B_idx", [NW, 128, SB * 8], mybir.dt.int16, isOutput=False)
    dstloc = P_("dstloc", [NW, 128, SW], f32, isOutput=False)
    dstrow = P_("dstrow", [NW, SW * 128], f32, isOutput=False)
    adst_widx = P_("adst_widx", [NW, 128, 1], mybir.dt.int32, isOutput=False)
    s2s_xidx = P_("s2s_xidx", [T, 128, 1], mybir.dt.int32, isOutput=False)
    s2s_bloc = P_("s2s_bloc", [T, 128, 1], f32, isOutput=False)
    s2s_brep = P_("s2s_brep", [T, 16, 128], f32, isOutput=False)
    WihT_a = P_("WihT_a", [128, 512], f32, isOutput=False)
    WihT_b = P_("WihT_b", [128, 512], f32, isOutput=False)
    WhhT = P_("WhhT", [128, 512], f32, isOutput=False)
    bg_rep = P_("bg_rep", [16, 512], f32, isOutput=False)
    W1a = P_("W1a", [128, 128], f32, isOutput=False)
    W1b = P_("W1b", [128, 128], f32, isOutput=False)
    W2 = P_("W2", [128, 128], f32, isOutput=False)
    b1_rep = P_("b1_rep", [16, 128], f32, isOutput=False)
    b2_rep = P_("b2_rep", [16, 128], f32, isOutput=False)
    out = P_("out", [16, 128], f32, isOutput=True)
    if DEBUG:
        dbg_xl = P_("dbg_xl", [256, ROWF], f32, isOutput=True)
        dbg_h1T = P_("dbg_h1T", [128, SHARD], f32, isOutput=True)
        dbg_h3 = P_("dbg_h3", [N, 128], f32, isOutput=True)
        dbg_ad = P_("dbg_ad", [128, 2], f32, isOutput=True)
        dbg_g0 = P_("dbg_g0", [128, ROWF], f32, isOutput=True)
        dbg_ex = P_("dbg_ex", [128, 2], f32, isOutput=True)
        dbg_adw = P_("dbg_adw", [128, 2], f32, isOutput=True)
        dbg_ade = P_("dbg_ade", [128, 2], f32, isOutput=True)
        dbg_pagg = P_("dbg_pagg", [128, 1024], f32, isOutput=True)
        dbg_m0 = P_("dbg_m0", [128, 128], f32, isOutput=True)

    xlA = nc.dram_tensor("xlA", [HALF, ROWF], f32)
    xlB = nc.dram_tensor("xlB", [HALF, ROWF], f32)
    adst_d = nc.dram_tensor("adst_d", [N, 2], f32)
    h_shT = nc.dram_tensor("h_shT", [128, SHARD], f32)
    ag_hT = nc.dram_tensor("ag_hT", [NCORES * 128, SHARD], f32,
                           addr_space="Shared")
    h_sh = nc.dram_tensor("h_sh", [SHARD, 128], f32)
    h3_full = nc.dram_tensor("h3_full", [N, 128], f32, addr_space="Shared")

    with tile.TileContext(nc) as tc:
        with tc.tile_pool(name="consts", bufs=1) as cp:
            ident = cp.tile([128, 128], f32)
            make_identity(nc, ident[:])
            iota_row = cp.tile([128, 128], f32)   # [p, j] = j
            nc.gpsimd.iota(iota_row[:], pattern=[[1, 128]], base=0,
                           channel_multiplier=0,
                           allow_small_or_imprecise_dtypes=True)
            iota_col = cp.tile([128, 1], f32)     # [p, 0] = p
            nc.gpsimd.iota(iota_col[:], pattern=[[0, 1]], base=0,
                           channel_multiplier=1,
                           allow_small_or_imprecise_dtypes=True)
            iota16_row = cp.tile([128, 16], f32)
            nc.gpsimd.iota(iota16_row[:], pattern=[[1, 16]], base=0,
                           channel_multiplier=0,
                           allow_small_or_imprecise_dtypes=True)
            iota16_col = cp.tile([16, 1], f32)
            nc.gpsimd.iota(iota16_col[:], pattern=[[0, 1]], base=0,
                           channel_multiplier=1,
                           allow_small_or_imprecise_dtypes=True)
            ones_row = cp.tile([1, 128], f32)
            nc.vector.memset(ones_row[:], 1.0)
            negones_row = cp.tile([1, 128], f32)
            nc.vector.memset(negones_row[:], -1.0)
            weff_sb = cp.tile([128, 262], f32)
            nc.sync.dma_start(out=weff_sb[:], in_=W_eff[:])
            bias_sb = cp.tile([128, 128], f32)
            nc.sync.dma_start(out=bias_sb[:], in_=bias_rep[:])
            wia = cp.tile([128, 512], f32)
            nc.sync.dma_start(out=wia[:], in_=WihT_a[:])
            wib = cp.tile([128, 512], f32)
            nc.sync.dma_start(out=wib[:], in_=WihT_b[:])
            whh = cp.tile([128, 512], f32)
            nc.sync.dma_start(out=whh[:], in_=WhhT[:])
            bg_sb = cp.tile([16, 512], f32)
            nc.sync.dma_start(out=bg_sb[:], in_=bg_rep[:])
            w1a_sb = cp.tile([128, 128], f32)
            nc.sync.dma_start(out=w1a_sb[:], in_=W1a[:])
            w1b_sb = cp.tile([128, 128], f32)
            nc.sync.dma_start(out=w1b_sb[:], in_=W1b[:])
            w2_sb = cp.tile([128, 128], f32)
            nc.sync.dma_start(out=w2_sb[:], in_=W2[:])
            b1_sb = cp.tile([16, 128], f32)
            nc.sync.dma_start(out=b1_sb[:], in_=b1_rep[:])
            b2_sb = cp.tile([16, 128], f32)
            nc.sync.dma_start(out=b2_sb[:], in_=b2_rep[:])

            for conv in range(NUM_CONVS):
                hT_src = h0T if conv == 0 else ag_hT
                # ---- phase 1: xl_ext = h @ W_eff for all N nodes ----
                with tc.tile_pool(name="p1s", bufs=3) as p1s, \
                     tc.tile_pool(name="p1p", bufs=3, space="PSUM") as p1p:
                    for s in range(NCORES):
                        xl_half, rbase = (xlA, SHARD * s) if s < 4 else \
                                         (xlB, SHARD * s - HALF)
                        for t in range(NW):
                            nwn = 128 if t < NW - 1 else LASTW
                            hT_t = p1s.tile([128, 128], f32, tag="hT")
                            nc.sync.dma_start(
                                out=hT_t[:, 0:nwn],
                                in_=hT_src[128 * s:128 * (s + 1),
                                           128 * t:128 * t + nwn])
                            ps = p1p.tile([128, 262], f32, tag="p1")
                            nc.tensor.matmul(ps[0:nwn, :], lhsT=hT_t[:, 0:nwn],
                                             rhs=weff_sb[:], start=True,
                                             stop=True)
                            xo = p1s.tile([128, 260], f32, tag="xo")
                            nc.scalar.copy(xo[0:nwn, :], ps[0:nwn, 0:260])
                            nc.vector.memset(
                                xo[0:nwn, 0:258].rearrange(
                                    "p (a b) -> p a b", b=129)[
                                    :, :, 128:129], 1.0)
                            ad = p1s.tile([128, 2], f32, tag="ad")
                            nc.vector.tensor_copy(ad[0:nwn, :],
                                                  ps[0:nwn, 260:262])
                            nc.sync.dma_start(
                                out=xl_half[rbase + 128 * t:
                                            rbase + 128 * t + nwn, 0:260],
                                in_=xo[0:nwn, :])
                            nc.sync.dma_start(
                                out=adst_d[SHARD * s + 128 * t:
                                           SHARD * s + 128 * t + nwn, :],
                                in_=ad[0:nwn, :])
                            if DEBUG and conv == 0 and s == 0 and t == 0:
                                nc.sync.dma_start(out=dbg_ad[:],
                                                  in_=ad[:])

                # ---- edge phase: one window of 128 dst nodes at a time ----
                with tc.tile_pool(name="eg", bufs=2) as eg, \
                     tc.tile_pool(name="es", bufs=2) as es, \
                     tc.tile_pool(name="em", bufs=3) as em, \
                     tc.tile_pool(name="agg", bufs=2, space="PSUM") as aggp, \
                     tc.tile_pool(name="drep", bufs=2, space="PSUM") as drp, \
                     tc.tile_pool(name="eap", bufs=1, space="PSUM") as eap, \
                     tc.tile_pool(name="etp", bufs=1, space="PSUM") as etp:
                    for w in range(NW):
                        nwn = 128 if w < NW - 1 else LASTW
                        gA = eg.tile([128, SA, ROWF], f32, tag="gA")
                        gB = eg.tile([128, SB, ROWF], f32, tag="gB")
                        iA = es.tile([128, SA * 8], mybir.dt.int16, tag="iA")
                        nc.sync.dma_start(out=iA[:], in_=A_idx[w])
                        iB = es.tile([128, SB * 8], mybir.dt.int16, tag="iB")
                        nc.sync.dma_start(out=iB[:], in_=B_idx[w])
                        q = 0
                        for (g, xh, S_h, it) in ((gA, xlA, SA, iA),
                                                 (gB, xlB, SB, iB)):
                            s0 = 0
                            while s0 < S_h:
                                ns = min(8, S_h - s0)
                                nc.gpsimd.dma_gather(
                                    out_ap=g[:, s0:s0 + ns, :], in_ap=xh[:],
                                    idxs_ap=it[:, s0 * 8:(s0 + ns) * 8],
                                    num_idxs=ns * 128, num_idxs_reg=ns * 128,
                                    elem_size=ROWF, queue_num=q % 4)
                                q += 1
                                s0 += ns
                        dl = es.tile([128, SW], f32, tag="dl")
                        nc.sync.dma_start(out=dl[:], in_=dstloc[w])
                        dr = es.tile([1, SW * 128], f32, tag="dr")
                        nc.sync.dma_start(out=dr[:], in_=dstrow[w:w + 1, :])
                        awi = es.tile([128, 1], mybir.dt.int32,
                                      tag="awi")
                        nc.sync.dma_start(out=awi[:], in_=adst_widx[w])
                        aw = es.tile([128, 2], f32, tag="aw")
                        nc.gpsimd.indirect_dma_start(
                            out=aw[:], out_offset=None, in_=adst_d[:],
                            in_offset=bass.IndirectOffsetOnAxis(
                                ap=awi[:], axis=0))
                        awsb = es.tile([128, 2], f32, tag="awsb")
                        nc.vector.tensor_copy(awsb[:], aw[:])

                        ngrp = (SW + 3) // 4
                        dreps = []
                        for gi in range(ngrp):
                            c0 = gi * 512
                            cn = min(512, SW * 128 - c0)
                            dt_ = drp.tile([128, 512], f32, tag="drep")
                            nc.tensor.matmul(dt_[:, 0:cn], lhsT=ones_row[:],
                                             rhs=dr[:, c0:c0 + cn],
                                             start=True, stop=True)
                            dreps.append(dt_)

                        pagg = aggp.tile([128, 1024], f32, tag="agg")
                        for s_ in range(SW):
                            g, si = (gA, s_) if s_ < SA else (gB, s_ - SA)
                            mt = em.tile([128, 128], f32, tag="mt")
                            nc.vector.tensor_scalar(
                                out=mt[:], in0=dreps[s_ // 4][
                                    :, (s_ % 4) * 128:(s_ % 4) * 128 + 128],
                                scalar1=iota_col[:], scalar2=None,
                                op0=OP.is_equal)
                            pe_ = eap.tile([128, 2], f32, tag="ae")
                            nc.tensor.matmul(pe_[:], lhsT=mt[:], rhs=awsb[:],
                                             start=True, stop=True)
                            lg = em.tile([128, 2], f32, tag="lg")
                            nc.vector.tensor_tensor(
                                out=lg[:], in0=g[:, si, 258:260], in1=pe_[:],
                                op=OP.add)
                            lr = em.tile([128, 2], f32, tag="lr")
                            nc.vector.scalar_tensor_tensor(
                                out=lr[:], in0=lg[:], scalar=NEG_SLOPE,
                                in1=lg[:], op0=OP.mult, op1=OP.max)
                            ex = em.tile([128, 2], f32, tag="ex")
                            nc.scalar.activation(ex[:], lr[:], AF.Exp)
                            m0 = em.tile([128, 128], f32, tag="m0")
                            nc.gpsimd.tensor_scalar(
                                out=m0[:], in0=iota_row[:],
                                scalar1=dl[:, s_:s_ + 1],
                                scalar2=ex[:, 0:1],
                                op0=OP.is_equal, op1=OP.mult)
                            m1 = em.tile([128, 128], f32, tag="m1")
                            nc.vector.tensor_scalar(
                                out=m1[:], in0=iota_row[:],
                                scalar1=dl[:, s_:s_ + 1],
                                scalar2=ex[:, 1:2],
                                op0=OP.is_equal, op1=OP.mult)
                            if DEBUG and conv == 0 and w == 0 and s_ == 9:
                                nc.sync.dma_start(out=dbg_g0[:],
                                                  in_=g[:, si, :])
                                nc.sync.dma_start(out=dbg_ex[:], in_=ex[:])
                                nc.sync.dma_start(out=dbg_adw[:], in_=awsb[:])
                                adcp = em.tile([128, 2], f32, tag="adcp")
                                nc.vector.tensor_copy(adcp[:], pe_[:])
                                nc.sync.dma_start(out=dbg_ade[:], in_=adcp[:])
                                nc.sync.dma_start(out=dbg_m0[:], in_=m0[:])
                            nc.tensor.matmul(pagg[:, 0:129], lhsT=m0[:],
                                             rhs=g[:, si, 0:129],
                                             start=(s_ == 0), stop=(s_ == SW - 1))
                            nc.tensor.matmul(pagg[:, 512:641], lhsT=m1[:],
                                             rhs=g[:, si, 129:258],
                                             start=(s_ == 0), stop=(s_ == SW - 1))

                        # combine: h_new = 0.5*(msg0*rs0 + msg1*rs1) + bias
                        if DEBUG and conv == 0 and w == 0:
                            pcp = em.tile([128, 1024], f32, tag="pcp")
                            nc.vector.tensor_copy(pcp[:], pagg[:])
                            nc.sync.dma_start(out=dbg_pagg[:], in_=pcp[:])
                        rs = em.tile([128, 2], f32, tag="rs")
                        nc.vector.tensor_scalar(
                            out=rs[:], in0=pagg[:].rearrange(
                                "p (a b) -> p a b", b=512)[:, :, 128:129],
                            scalar1=1e-16, scalar2=None, op0=OP.add)
                        nc.vector.reciprocal(rs[:], rs[:])
                        nc.vector.tensor_scalar(out=rs[:], in0=rs[:],
                                                scalar1=0.5, scalar2=None,
                                                op0=OP.mult)
                        t0 = em.tile([128, 128], f32, tag="t0")
                        nc.vector.tensor_scalar(out=t0[:], in0=pagg[:, 0:128],
                                                scalar1=rs[:, 0:1],
                                                scalar2=None, op0=OP.mult)
                        t1 = em.tile([128, 128], f32, tag="t1")
                        nc.vector.tensor_scalar(out=t1[:],
                                                in0=pagg[:, 512:640],
                                                scalar1=rs[:, 1:2],
                                                scalar2=None, op0=OP.mult)
                        hn = em.tile([128, 128], f32, tag="hn")
                        nc.vector.tensor_tensor(out=hn[:], in0=t0[:],
                                                in1=t1[:], op=OP.add)
                        nc.vector.tensor_tensor(out=hn[:], in0=hn[:],
                                                in1=bias_sb[:], op=OP.add)
                        if conv < NUM_CONVS - 1:
                            pt = etp.tile([128, 128], f32, tag="pt")
                            nc.tensor.transpose(pt[:], hn[:], ident[:])
                            ht = em.tile([128, 128], f32, tag="ht")
                            nc.vector.tensor_copy(ht[:], pt[:])
                            nc.sync.dma_start(
                                out=h_shT[:, 128 * w:128 * w + nwn],
                                in_=ht[:, 0:nwn])
                        else:
                            nc.sync.dma_start(
                                out=h_sh[128 * w:128 * w + nwn, :],
                                in_=hn[0:nwn, :])

                if DEBUG and conv == 0:
                    nc.sync.dma_start(out=dbg_xl[0:128, :],
                                      in_=xlA[0:128, :])
                    nc.sync.dma_start(out=dbg_xl[128:256, :],
                                      in_=xlA[HALF - 128:HALF, :])
                    nc.sync.dma_start(out=dbg_h1T[:], in_=h_shT[:])
                if conv < NUM_CONVS - 1:
                    nc.gpsimd.collective_compute(
                        "AllGather", mybir.AluOpType.bypass,
                        ins=[h_shT[:]], outs=[ag_hT[:]],
                        replica_groups=[list(range(NCORES))])
                else:
                    nc.gpsimd.collective_compute(
                        "AllGather", mybir.AluOpType.bypass,
                        ins=[h_sh[:]], outs=[h3_full[:]],
                        replica_groups=[list(range(NCORES))])
                    if DEBUG:
                        nc.sync.dma_start(out=dbg_h3[:], in_=h3_full[:])

            # ---- set2set on this core's 16-graph slice ----
            with tc.tile_pool(name="s2s", bufs=1) as sp, \
                 tc.tile_pool(name="s2w", bufs=2) as swp, \
                 tc.tile_pool(name="s2p", bufs=2, space="PSUM") as s2p, \
                 tc.tile_pool(name="s2g", bufs=1, space="PSUM") as s2g:
                xloc = sp.tile([128, T, 128], f32)
                xidx_sb = sp.tile([128, T], mybir.dt.int32)
                nc.sync.dma_start(
                    out=xidx_sb[:],
                    in_=s2s_xidx.rearrange("t p o -> p (t o)"))
                for t in range(T):
                    nc.gpsimd.indirect_dma_start(
                        out=xloc[:, t, :], out_offset=None, in_=h3_full[:],
                        in_offset=bass.IndirectOffsetOnAxis(
                            ap=xidx_sb[:, t:t + 1], axis=0))
                bl = sp.tile([128, T], f32)
                nc.sync.dma_start(out=bl[:],
                                  in_=s2s_bloc.rearrange("t p o -> p (t o)"))
                brep_sb = sp.tile([16, T, 128], f32)
                nc.sync.dma_start(out=brep_sb[:],
                                  in_=s2s_brep.rearrange("t p d -> p t d"))
                oh = sp.tile([128, T, 16], f32)
                ohT = sp.tile([16, T, 128], f32)
                for t in range(T):
                    nc.vector.tensor_scalar(
                        out=oh[:, t, :], in0=iota16_row[:],
                        scalar1=bl[:, t:t + 1], scalar2=None, op0=OP.is_equal)
                    nc.vector.tensor_scalar(
                        out=ohT[:, t, :], in0=brep_sb[:, t, :],
                        scalar1=iota16_col[:], scalar2=None, op0=OP.is_equal)

                qT = sp.tile([128, 16], f32)
                nc.vector.memset(qT[:], 0.0)
                rT = sp.tile([128, 16], f32)
                nc.vector.memset(rT[:], 0.0)
                cst = sp.tile([16, 128], f32)
                nc.vector.memset(cst[:], 0.0)
                eloc = sp.tile([128, T], f32)

                for step in range(AGGR_STEPS):
                    pg = s2g.tile([16, 512], f32, tag="acc")
                    nc.tensor.matmul(pg[:], lhsT=qT[:], rhs=wia[:],
                                     start=True, stop=False)
                    nc.tensor.matmul(pg[:], lhsT=rT[:], rhs=wib[:],
                                     start=False, stop=False)
                    nc.tensor.matmul(pg[:], lhsT=qT[:], rhs=whh[:],
                                     start=False, stop=True)
                    gt = swp.tile([16, 512], f32, tag="gt")
                    nc.vector.tensor_tensor(out=gt[:], in0=pg[:], in1=bg_sb[:],
                                            op=OP.add)
                    sf = swp.tile([16, 128], f32, tag="sf")
                    nc.scalar.activation(sf[:], gt[:, 128:256], AF.Sigmoid)
                    si_ = swp.tile([16, 128], f32, tag="si")
                    nc.scalar.activation(si_[:], gt[:, 0:128], AF.Sigmoid)
                    tg = swp.tile([16, 128], f32, tag="tg")
                    nc.scalar.activation(tg[:], gt[:, 256:384], AF.Tanh)
                    so = swp.tile([16, 128], f32, tag="so")
                    nc.scalar.activation(so[:], gt[:, 384:512], AF.Sigmoid)
                    c2 = swp.tile([16, 128], f32, tag="c2")
                    nc.vector.tensor_tensor(out=c2[:], in0=sf[:], in1=cst[:],
                                            op=OP.mult)
                    it_ = swp.tile([16, 128], f32, tag="it")
                    nc.vector.tensor_tensor(out=it_[:], in0=si_[:], in1=tg[:],
                                            op=OP.mult)
                    nc.vector.tensor_tensor(out=c2[:], in0=c2[:], in1=it_[:],
                                            op=OP.add)
                    nc.vector.tensor_copy(cst[:], c2[:])
                    tc2 = swp.tile([16, 128], f32, tag="tc2")
                    nc.scalar.activation(tc2[:], c2[:], AF.Tanh)
                    qpad = swp.tile([128, 128], f32, tag="qpad")
                    nc.vector.memset(qpad[:], 0.0)
                    nc.vector.tensor_tensor(out=qpad[0:16, :], in0=so[:],
                                            in1=tc2[:], op=OP.mult)
                    ptq = s2p.tile([128, 128], f32, tag="tp")
                    nc.tensor.transpose(ptq[:], qpad[:], ident[:])
                    nc.vector.tensor_copy(qT[:], ptq[:, 0:16])

                    # e_n = x_n . q[batch_n]
                    for t in range(T):
                        pqx = s2p.tile([128, 128], f32, tag="tp")
                        nc.tensor.matmul(pqx[:], lhsT=ohT[:, t, :],
                                         rhs=qpad[0:16, :], start=True,
                                         stop=True)
                        xq = swp.tile([128, 128], f32, tag="xq")
                        nc.vector.tensor_tensor(out=xq[:], in0=xloc[:, t, :],
                                                in1=pqx[:], op=OP.mult)
                        nc.vector.tensor_reduce(
                            out=eloc[:, t:t + 1], in_=xq[:],
                            axis=mybir.AxisListType.X, op=OP.add)
                    # global (per-core) max for stability
                    mx = swp.tile([128, 1], f32, tag="mx")
                    nc.vector.tensor_reduce(out=mx[:], in_=eloc[:],
                                            axis=mybir.AxisListType.X,
                                            op=OP.max)
                    mpad = swp.tile([128, 128], f32, tag="mpad")
                    nc.vector.memset(mpad[:], -1e30)
                    nc.vector.tensor_copy(mpad[:, 0:1], mx[:])
                    ptm = s2p.tile([128, 128], f32, tag="tp")
                    nc.tensor.transpose(ptm[:], mpad[:], ident[:])
                    msc = swp.tile([1, 1], f32, tag="msc")
                    nc.vector.tensor_reduce(out=msc[:], in_=ptm[0:1, :],
                                            axis=mybir.AxisListType.X,
                                            op=OP.max)
                    pnm = s2p.tile([128, 1], f32, tag="tp")
                    nc.tensor.matmul(pnm[:], lhsT=negones_row[:], rhs=msc[:],
                                     start=True, stop=True)
                    negm = swp.tile([128, 1], f32, tag="negm")
                    nc.vector.tensor_copy(negm[:], pnm[:])

                    pr = s2g.tile([16, 129], f32, tag="acc")
                    for t in range(T):
                        ev = swp.tile([128, 1], f32, tag="ev")
                        nc.scalar.activation(ev[:], eloc[:, t:t + 1], AF.Exp,
                                             bias=negm[:, 0:1])
                        msg = swp.tile([128, 129], f32, tag="msg")
                        nc.scalar.activation(msg[:, 0:128], xloc[:, t, :],
                                             AF.Copy, scale=ev[:, 0:1])
                        nc.vector.tensor_copy(msg[:, 128:129], ev[:])
                        nc.tensor.matmul(pr[:], lhsT=oh[:, t, :], rhs=msg[:],
                                         start=(t == 0), stop=(t == T - 1))
                    rsum = swp.tile([16, 1], f32, tag="rsum")
                    nc.vector.tensor_scalar(out=rsum[:], in0=pr[:, 128:129],
                                            scalar1=1e-16, scalar2=None,
                                            op0=OP.add)
                    nc.vector.reciprocal(rsum[:], rsum[:])
                    rpad = swp.tile([128, 128], f32, tag="rpad")
                    nc.vector.memset(rpad[:], 0.0)
                    nc.vector.tensor_scalar(out=rpad[0:16, :],
                                            in0=pr[:, 0:128],
                                            scalar1=rsum[:, 0:1],
                                            scalar2=None, op0=OP.mult)
                    ptr = s2p.tile([128, 128], f32, tag="tp")
                    nc.tensor.transpose(ptr[:], rpad[:], ident[:])
                    nc.vector.tensor_copy(rT[:], ptr[:, 0:16])

                # MLP head
                pm1 = s2g.tile([16, 128], f32, tag="acc")
                nc.tensor.matmul(pm1[:], lhsT=qT[:], rhs=w1a_sb[:],
                                 start=True, stop=False)
                nc.tensor.matmul(pm1[:], lhsT=rT[:], rhs=w1b_sb[:],
                                 start=False, stop=True)
                hidp = swp.tile([128, 128], f32, tag="hidp")
                nc.vector.memset(hidp[:], 0.0)
                nc.vector.tensor_tensor(out=hidp[0:16, :], in0=pm1[:],
                                        in1=b1_sb[:], op=OP.add)
                nc.scalar.activation(hidp[0:16, :], hidp[0:16, :], AF.Relu)
                pth = s2p.tile([128, 128], f32, tag="tp")
                nc.tensor.transpose(pth[:], hidp[:], ident[:])
                hT_m = swp.tile([128, 16], f32, tag="hTm")
                nc.vector.tensor_copy(hT_m[:], pth[:, 0:16])
                pm2 = s2g.tile([16, 128], f32, tag="acc")
                nc.tensor.matmul(pm2[:], lhsT=hT_m[:], rhs=w2_sb[:],
                                 start=True, stop=True)
                osb = swp.tile([16, 128], f32, tag="osb")
                nc.vector.tensor_tensor(out=osb[:], in0=pm2[:], in1=b2_sb[:],
                                        op=OP.add)
                nc.sync.dma_start(out=out[:], in_=osb[:])

    nc.compile()
    _split_waits(nc)
    return nc


# ---------------------------------------------------------------- entry
def kernel(x, edge_index, edge_attr, batch_index,
           gat_W, gat_att_src, gat_att_dst, gat_bias,
           lstm_Wih, lstm_Whh, lstm_bih, lstm_bhh,
           mlp_W1, mlp_b1, mlp_W2, mlp_b2, _trace=False):
    del edge_attr
    x = np.asarray(x, np.float32)
    edge_index = np.asarray(edge_index)
    batch_index = np.asarray(batch_index)

    cfg, per_core = _host_prep(x, edge_index, batch_index,
                               gat_W, gat_att_src, gat_att_dst)

    Wih = np.asarray(lstm_Wih, np.float32)     # [512, 256]
    Whh = np.asarray(lstm_Whh, np.float32)     # [512, 128]
    WihT = Wih.T.copy()                        # [256, 512]
    bias_gates = (np.asarray(lstm_bih, np.float32)
                  + np.asarray(lstm_bhh, np.float32))
    common = dict(
        h0T=cfg["h0T"], W_eff=cfg["W_eff"],
        bias_rep=np.tile(np.asarray(gat_bias, np.float32)[None, :],
                         (128, 1)),
        WihT_a=WihT[0:128], WihT_b=WihT[128:256],
        WhhT=Whh.T.copy(),
        bg_rep=np.tile(bias_gates[None, :], (16, 1)),
        W1a=np.asarray(mlp_W1, np.float32)[0:128],
        W1b=np.asarray(mlp_W1, np.float32)[128:256],
        W2=np.asarray(mlp_W2, np.float32),
        b1_rep=np.tile(np.asarray(mlp_b1, np.float32)[None, :], (16, 1)),
        b2_rep=np.tile(np.asarray(mlp_b2, np.float32)[None, :], (16, 1)),
    )

    key = (cfg["SA"], cfg["SB"], cfg["T"])
    if _cached.get("key") != key:
        _cached["nc"] = _build(cfg)
        _cached["key"] = key
    nc = _cached["nc"]

    in_maps = []
    for c in range(NCORES):
        m = dict(common)
        m.update(per_core[c])
        m = {k: np.ascontiguousarray(v) for k, v in m.items()}
        in_maps.append(m)

    from concourse.bass_utils import run_bass_kernel_spmd
    res = run_bass_kernel_spmd(nc, in_maps, core_ids=list(range(NCORES)),
                               trace=_trace)
    outp = np.concatenate([res.results[c]["out"] for c in range(NCORES)],
                          axis=0)
    if _trace:
        _cached["last_exec_ns"] = res.exec_time_ns
        _cached["last_res"] = res
    return outp



# revision 22
# speedup vs baseline: 2.8014x; 2.8014x over previous
"""GAT (3 convs) + Set2Set + MLP on 8 Trainium2 NeuronCores — v2.

Sharding: nodes in 8 ranges of 6250; edges (incl self-loops) sorted by dst,
sharded by dst range; per-dst segment softmax is core-local.

v2 design (vs v1): all conv traffic in bf16; one-hot edge->dst masks are
precomputed on host (Mpure [e,j] and MpureT [j,e] per 128-edge slab) and
DMA-loaded instead of built per-slab with slow tensor_scalar ops; the
leakyrelu/exp logit chain is batched per 128-dst window; message aggregation
is one bf16 matmul per slab (lhsT=Mpure, rhs = per-edge rows scaled by
exp(logit) per head: DVE scales head0, Scalar head1); a_dst broadcast to
edges via matmul with MpureT.  Gathered row layout (bf16, 384 cols = 768 B):
[xl0(128) | 1 | xl1(128) | 1 | asrc(2) | pad].  Phase 1 loads whole-shard hT
tiles and writes xl in shard-sized batches.  Set2Set+MLP unchanged (fp32).
"""
import os
import sys

import numpy as np
import ml_dtypes

sys.path.insert(0, "/opt/trn_rl_repo")

BF = ml_dtypes.bfloat16

N, E, F_RAW, D, H, B = 50000, 800000, 9, 128, 2, 128
NUM_CONVS = int(os.environ.get("K_CONVS", "3"))
AGGR_STEPS = int(os.environ.get("K_STEPS", "3"))
NEG_SLOPE = 0.2
NCORES = 8
SHARD = N // NCORES            # 6250
HALF = N // 2                  # 25000
NW = (SHARD + 127) // 128      # 49 windows per core
LASTW = SHARD - (NW - 1) * 128 # 106
ROWC = 384                     # gathered xl row, bf16 cols (768B, %256==0)
GW = 2                         # windows per DMA load-group

_cached = {}


# ---------------------------------------------------------------- patches
def _install_patches():
    import concourse.tile as tile_mod
    from concourse.vector_clock import ScopedClock, VectorClock

    if not getattr(tile_mod.TileContext, "_drain_patched", False):
        def patched(self, tick_clock, wait_clock):
            gc = tick_clock.global_clock
            vals = [gc[p] for p in range(27)]
            for p in [p for p in range(27) if vals[p] > 0]:
                sub = [vals[q] if q == p else 0 for q in range(27)]
                nop = self.nc.sync.nop(nofuse=True, hint="drain_wait_split")
                wait_clock.add_sem_waits(
                    nop.ins, ScopedClock({None: VectorClock(sub)}))
            self.nc.sync.drain()
            self.nc.all_engine_barrier()
            popped = self.nc._tile_sem_poison_stack.pop()
            assert popped is self._sem_poison
            self.nc.clear_and_free_semaphores(
                list(self.sems.allocated().values()))
            self.nc.all_engine_barrier()

        tile_mod.TileContext._drain_and_barrier = patched
        tile_mod.TileContext._drain_patched = True


def _split_waits(nc, max_waits=1):
    """walrus allows at most one sync-wait command per instruction;
    spread extras across injected same-engine NoOps."""
    from concourse import mybir
    n = 0
    for f in nc.m.functions:
        for bb in f.blocks:
            changed, new = False, []
            for ins in bb.instructions:
                si = ins.sync_info
                if si is not None and len(si.on_wait) > max_waits:
                    waits = list(si.on_wait)
                    for i, w in enumerate(waits[max_waits:]):
                        nop = mybir.InstNoOp(
                            name=f"{ins.name}-ws{i}", ins=[], outs=[])
                        nop.engine = ins.engine
                        nop.sync_info = mybir.SyncInfo(
                            on_wait=[w], on_update=[])
                        new.append(nop)
                    ins.sync_info = mybir.SyncInfo(
                        on_wait=waits[:max_waits],
                        on_update=list(si.on_update))
                    changed = True
                    n += 1
                new.append(ins)
            if changed:
                bb.instructions = new
    return n


# ---------------------------------------------------------------- host prep
def _wrap16(flat):
    """dma_gather index layout: idx k at [k%16, k//16], replicated to 128."""
    k = flat.shape[0]
    w = flat.reshape(k // 16, 16).T.astype(np.int16)
    return np.tile(w, (8, 1))


def _pack(src, dst):
    """Permute each core's nodes into windows balancing per-window edge
    counts, and split nodes across the two xl tensors (exactly 64 A-slots
    per full window, 53 in the last) balancing per-window per-half counts.
    Returns pos[n] (global permuted position), ha[n] (0=xlA, 1=xlB)."""
    din = np.zeros(N, np.int64)
    np.add.at(din, dst, 1)
    win_of_node = np.empty(N, np.int64)     # global window id
    for c in range(NCORES):
        nodes = np.arange(SHARD * c, SHARD * (c + 1))
        d = din[nodes]
        order = np.argsort(-d, kind="stable")
        caps = np.full(NW, 128, np.int64)
        caps[-1] = LASTW
        loads = np.zeros(NW, np.int64)
        sizes = np.zeros(NW, np.int64)
        capw = caps / caps.sum()
        for i in order:
            score = (loads + d[i]) / capw
            score[sizes >= caps] = np.inf
            g = int(np.argmin(score))
            win_of_node[nodes[i]] = c * NW + g
            sizes[g] += 1
            loads[g] += d[i]

    GTOT = NCORES * NW
    acap = np.full(GTOT, 64, np.int64)
    acap[NW - 1::NW] = 53
    bcap = np.full(GTOT, 128, np.int64)
    bcap[NW - 1::NW] = LASTW
    bcap -= acap
    eorder = np.argsort(src, kind="stable")
    ew = win_of_node[dst[eorder]]
    estart = np.searchsorted(src[eorder], np.arange(N + 1))
    outdeg = np.diff(estart)
    imb = np.zeros(GTOT, np.float64)
    acnt = np.zeros(GTOT, np.int64)
    bcnt = np.zeros(GTOT, np.int64)
    ha = np.empty(N, np.int8)
    for n in np.argsort(-outdeg, kind="stable"):
        hw = win_of_node[n]
        wl = ew[estart[n]:estart[n + 1]]
        prefer_a = imb[wl].sum() < 0
        if acnt[hw] < acap[hw] and (prefer_a or bcnt[hw] >= bcap[hw]):
            ha[n] = 0
            acnt[hw] += 1
            imb[wl] += 0.5
        else:
            ha[n] = 1
            bcnt[hw] += 1
            imb[wl] -= 0.5

    pos = np.empty(N, np.int64)
    base = np.zeros(GTOT + 1, np.int64)
    np.cumsum(np.where(np.arange(GTOT) % NW == NW - 1, LASTW, 128),
              out=base[1:])
    afill = np.zeros(GTOT, np.int64)
    bfill = acap.copy()
    for n in range(N):
        g = win_of_node[n]
        if ha[n] == 0:
            pos[n] = base[g] + afill[g]
            afill[g] += 1
        else:
            pos[n] = base[g] + bfill[g]
            bfill[g] += 1
    return pos, ha, acap


def _host_prep(x, edge_index, batch_index, gat_W, gat_att_src, gat_att_dst):
    cfg = {}
    src = np.concatenate([edge_index[0], np.arange(N, dtype=np.int64)])
    dst = np.concatenate([edge_index[1], np.arange(N, dtype=np.int64)])
    order = np.argsort(dst, kind="stable")
    src, dst = src[order].astype(np.int32), dst[order].astype(np.int32)
    if os.environ.get("K_SAVE_EDGES"):
        np.savez("/tmp/edges.npz", src=src, dst=dst)

    pos, ha, acap = _pack(src, dst)
    GTOT = NCORES * NW
    base = np.zeros(GTOT + 1, np.int64)
    np.cumsum(np.where(np.arange(GTOT) % NW == NW - 1, LASTW, 128),
              out=base[1:])
    abase = np.zeros(GTOT + 1, np.int64)
    np.cumsum(acap, out=abase[1:])
    bcapg = np.where(np.arange(GTOT) % NW == NW - 1, LASTW, 128) - acap
    bbase = np.zeros(GTOT + 1, np.int64)
    np.cumsum(bcapg, out=bbase[1:])
    g_of = (pos // SHARD) * NW + (pos % SHARD) // 128
    loc = pos - base[g_of]
    row = np.where(ha == 0, abase[g_of] + loc,
                   bbase[g_of] + (loc - acap[g_of]))

    # per (core, window, half): edge lists keyed by PERMUTED dst position
    pdst = pos[dst]
    core_of = pdst // SHARD
    win_of = (pdst % SHARD) // 128
    half_of = ha[src].astype(np.int32)
    key = ((core_of * NW + win_of) * 2 + half_of)
    korder = np.argsort(key, kind="stable")
    src_s, dst_s = src[korder], dst[korder]
    counts = np.bincount(key[korder], minlength=NCORES * NW * 2).reshape(
        NCORES, NW, 2)
    SA = int(np.ceil(counts[:, :, 0].max() / 128))
    SB = int(np.ceil(counts[:, :, 1].max() / 128))
    SW = SA + SB
    cfg["SA"], cfg["SB"], cfg["SW"] = SA, SB, SW

    starts = np.zeros(NCORES * NW * 2 + 1, np.int64)
    np.cumsum(counts.reshape(-1), out=starts[1:])

    # graph boundaries for set2set
    goff = np.searchsorted(batch_index, np.arange(B + 1))
    rows_per_core = np.array(
        [goff[16 * (c + 1)] - goff[16 * c] for c in range(NCORES)])
    T = int(np.ceil(rows_per_core.max() / 128))
    cfg["T"] = T

    # replicated weights: W_eff bf16 [128, 264]
    W = np.asarray(gat_W, np.float32)              # [128, 256]
    asrc_v = np.asarray(gat_att_src, np.float32)   # [2, 128]
    adst_v = np.asarray(gat_att_dst, np.float32)
    w_as = np.stack([W[:, h * D:(h + 1) * D] @ asrc_v[h] for h in range(H)],
                    axis=1)                        # [128, 2]
    w_ad = np.stack([W[:, h * D:(h + 1) * D] @ adst_v[h] for h in range(H)],
                    axis=1)
    W_eff = np.zeros((D, 264), np.float32)
    W_eff[:, 0:128] = W[:, 0:128]
    W_eff[:, 129:257] = W[:, 128:256]
    W_eff[:, 258:260] = w_as
    W_eff[:, 260:262] = w_ad
    cfg["W_eff"] = W_eff.astype(BF)

    xp = np.zeros((N, D), np.float32)
    xp[:, :F_RAW] = x
    invpos = np.empty(N, np.int64)
    invpos[pos] = np.arange(N)
    h0T = np.zeros((NCORES * 128, SHARD), np.float32)
    for s in range(NCORES):
        h0T[128 * s:128 * (s + 1), :] = \
            xp[invpos[SHARD * s:SHARD * (s + 1)]].T
    cfg["h0T"] = h0T.astype(BF)

    per_core = []
    e_rng = np.arange(128)
    for c in range(NCORES):
        A_idx = np.zeros((NW, 128, SA * 8), np.int16)
        B_idx = np.zeros((NW, 128, SB * 8), np.int16)
        Mp = np.zeros((NW, 128, SW * 128), BF)     # [w, e, si*128 + j]
        MpT = np.zeros((NW, 128, SW * 128), BF)    # [w, j, si*128 + e]
        for w in range(NW):
            for hf, (idx_arr, S_h) in enumerate(((A_idx, SA), (B_idx, SB))):
                k = (c * NW + w) * 2 + hf
                lo, hi = starts[k], starts[k + 1]
                cnt = hi - lo
                flat = np.zeros(S_h * 128, np.int64)  # pad -> row 0 (finite)
                flat[:cnt] = row[src_s[lo:hi]]
                idx_arr[w] = _wrap16(flat)
                dl = np.full(S_h * 128, -1, np.int64)
                dl[:cnt] = pos[dst_s[lo:hi]] % SHARD - 128 * w
                base = 0 if hf == 0 else SA
                for s_ in range(S_h):
                    dls = dl[s_ * 128:(s_ + 1) * 128]
                    v = dls >= 0
                    si = base + s_
                    Mp[w, e_rng[v], si * 128 + dls[v]] = 1
                    MpT[w, dls[v], si * 128 + e_rng[v]] = 1

        adst_widx = np.zeros((NW, 128, 1), np.int32)
        for w in range(NW):
            rows = SHARD * c + 128 * w + np.arange(128)
            adst_widx[w, :, 0] = np.minimum(rows, N - 1)

        # set2set slice
        r0, r1 = goff[16 * c], goff[16 * (c + 1)]
        xidx = np.zeros((T, 128, 1), np.int32)
        bloc = np.full((T, 128, 1), -1.0, np.float32)
        rows = np.arange(T * 128)
        glob = np.minimum(r0 + rows, N - 1)
        xidx[:, :, 0] = pos[glob].reshape(T, 128)
        valid = (r0 + rows) < r1
        bl = np.full(T * 128, -1.0, np.float32)
        bl[valid] = (batch_index[glob[valid]] - 16 * c).astype(np.float32)
        bloc[:, :, 0] = bl.reshape(T, 128)
        brep = np.tile(bl.reshape(T, 1, 128), (1, 16, 1)).astype(np.float32)

        per_core.append(dict(
            A_idx=A_idx, B_idx=B_idx, Mpure=Mp, MpureT=MpT,
            adst_widx=adst_widx,
            s2s_xidx=xidx, s2s_bloc=bloc, s2s_brep=brep,
        ))
    return cfg, per_core


# ---------------------------------------------------------------- device build
def _build(cfg):
    import concourse.bacc as bacc
    import concourse.bass as bass
    import concourse.tile as tile
    from concourse import mybir
    from concourse.masks import make_identity

    _install_patches()
    f32 = mybir.dt.float32
    bf16 = mybir.dt.bfloat16
    AF = mybir.ActivationFunctionType
    OP = mybir.AluOpType
    SA, SB, SW, T = cfg["SA"], cfg["SB"], cfg["SW"], cfg["T"]

    nc = bacc.Bacc("TRN2", num_swdge_queues=4)
    P_ = nc.declare_dram_parameter
    h0T = P_("h0T", [NCORES * 128, SHARD], bf16, isOutput=False)
    W_eff = P_("W_eff", [D, 264], bf16, isOutput=False)
    bias_rep = P_("bias_rep", [128, 128], f32, isOutput=False)
    A_idx = P_("A_idx", [NW, 128, SA * 8], mybir.dt.int16, isOutput=False)
    B_idx = P_("B_idx", [NW, 128, SB * 8], mybir.dt.int16, isOutput=False)
    Mpure = P_("Mpure", [NW, 128, SW * 128], bf16, isOutput=False)
    MpureT = P_("MpureT", [NW, 128, SW * 128], bf16, isOutput=False)
    adst_widx = P_("adst_widx", [NW, 128, 1], mybir.dt.int32, isOutput=False)
    s2s_xidx = P_("s2s_xidx", [T, 128, 1], mybir.dt.int32, isOutput=False)
    s2s_bloc = P_("s2s_bloc", [T, 128, 1], f32, isOutput=False)
    s2s_brep = P_("s2s_brep", [T, 16, 128], f32, isOutput=False)
    WihT_a = P_("WihT_a", [128, 512], f32, isOutput=False)
    WihT_b = P_("WihT_b", [128, 512], f32, isOutput=False)
    WhhT = P_("WhhT", [128, 512], f32, isOutput=False)
    bg_rep = P_("bg_rep", [16, 512], f32, isOutput=False)
    W1a = P_("W1a", [128, 128], f32, isOutput=False)
    W1b = P_("W1b", [128, 128], f32, isOutput=False)
    W2 = P_("W2", [128, 128], f32, isOutput=False)
    b1_rep = P_("b1_rep", [16, 128], f32, isOutput=False)
    b2_rep = P_("b2_rep", [16, 128], f32, isOutput=False)
    out = P_("out", [16, 128], f32, isOutput=True)

    if os.environ.get("K_DEBUG"):
        xlA = P_("xlA", [HALF, ROWC], bf16, isOutput=True)
        xlB = P_("xlB", [HALF, ROWC], bf16, isOutput=True)
        adst_d = P_("adst_d", [N, 2], bf16, isOutput=True)
    else:
        xlA = nc.dram_tensor("xlA", [HALF, ROWC], bf16)
        xlB = nc.dram_tensor("xlB", [HALF, ROWC], bf16)
        adst_d = nc.dram_tensor("adst_d", [N, 2], bf16)
    h_shT = nc.dram_tensor("h_shT", [128, SHARD], bf16)
    ag_hT = nc.dram_tensor("ag_hT", [NCORES * 128, SHARD], bf16,
                           addr_space="Shared")
    h_sh = nc.dram_tensor("h_sh", [SHARD, 128], f32)
    h3_full = nc.dram_tensor("h3_full", [N, 128], f32, addr_space="Shared")

    # Pool-engine DMA instructions get DMASW sem lanes round-robin and each
    # lane is locked to ONE SWDGE queue.  indirect_dma (qPoolDynamic ~= q0)
    # shares those lanes, so every gpsimd DMA must use queue 0.
    def swdge_q():
        return 0

    with tile.TileContext(nc) as tc:
        with tc.tile_pool(name="consts", bufs=1) as cp:
            ident = cp.tile([128, 128], f32)
            make_identity(nc, ident[:])
            iota16_row = cp.tile([128, 16], f32)
            nc.gpsimd.iota(iota16_row[:], pattern=[[1, 16]], base=0,
                           channel_multiplier=0,
                           allow_small_or_imprecise_dtypes=True)
            iota16_col = cp.tile([16, 1], f32)
            nc.gpsimd.iota(iota16_col[:], pattern=[[0, 1]], base=0,
                           channel_multiplier=1,
                           allow_small_or_imprecise_dtypes=True)
            negones_row = cp.tile([1, 128], f32)
            nc.vector.memset(negones_row[:], -1.0)
            weff_sb = cp.tile([128, 264], bf16)
            nc.sync.dma_start(out=weff_sb[:], in_=W_eff[:])
            bias_sb = cp.tile([128, 128], f32)
            nc.sync.dma_start(out=bias_sb[:], in_=bias_rep[:])
            wia = cp.tile([128, 512], f32)
            nc.sync.dma_start(out=wia[:], in_=WihT_a[:])
            wib = cp.tile([128, 512], f32)
            nc.sync.dma_start(out=wib[:], in_=WihT_b[:])
            whh = cp.tile([128, 512], f32)
            nc.sync.dma_start(out=whh[:], in_=WhhT[:])
            bg_sb = cp.tile([16, 512], f32)
            nc.sync.dma_start(out=bg_sb[:], in_=bg_rep[:])
            w1a_sb = cp.tile([128, 128], f32)
            nc.sync.dma_start(out=w1a_sb[:], in_=W1a[:])
            w1b_sb = cp.tile([128, 128], f32)
            nc.sync.dma_start(out=w1b_sb[:], in_=W1b[:])
            w2_sb = cp.tile([128, 128], f32)
            nc.sync.dma_start(out=w2_sb[:], in_=W2[:])
            b1_sb = cp.tile([16, 128], f32)
            nc.sync.dma_start(out=b1_sb[:], in_=b1_rep[:])
            b2_sb = cp.tile([16, 128], f32)
            nc.sync.dma_start(out=b2_sb[:], in_=b2_rep[:])

            for conv in range(NUM_CONVS):
                hT_src = h0T if conv == 0 else ag_hT
                # ---- phase 1: xl = h @ W_eff for all N nodes (bf16) ----
                with tc.tile_pool(name="p1h", bufs=2) as p1h, \
                     tc.tile_pool(name="p1w", bufs=2) as p1w, \
                     tc.tile_pool(name="p1p", bufs=4, space="PSUM") as p1p:
                    for s in range(NCORES):
                        hT_t = p1h.tile([128, SHARD], bf16, tag="hT")
                        nc.sync.dma_start(
                            out=hT_t[:],
                            in_=hT_src[128 * s:128 * (s + 1), :])
                        xo = p1w.tile([128, NW, 264], bf16, tag="xo")
                        ad = p1w.tile([128, NW, 2], bf16, tag="ad")
                        for t in range(NW):
                            nwn = 128 if t < NW - 1 else LASTW
                            ps = p1p.tile([128, 262], f32, tag="p1")
                            nc.tensor.matmul(
                                ps[0:nwn, :],
                                lhsT=hT_t[:, 128 * t:128 * t + nwn],
                                rhs=weff_sb[:, 0:262], start=True, stop=True)
                            eng = nc.vector if t % 2 == 0 else nc.scalar
                            if t % 2 == 0:
                                nc.vector.tensor_copy(xo[0:nwn, t, 0:260],
                                                      ps[0:nwn, 0:260])
                            else:
                                nc.scalar.copy(xo[0:nwn, t, 0:260],
                                               ps[0:nwn, 0:260])
                            nc.vector.memset(
                                xo[0:nwn, t, 0:258].rearrange(
                                    "p (a b) -> p a b", b=129)[:, :, 128:129],
                                1.0)
                            nc.vector.tensor_copy(ad[0:nwn, t, :],
                                                  ps[0:nwn, 260:262])
                        # xl rows split across xlA/xlB: window partitions
                        # [0:64) -> xlA, [64:128) -> xlB (last window 53/53);
                        # each shard owns 3125 contiguous rows in each half
                        a0 = 3125 * s
                        nfa = (NW - 1) * 64      # 3072
                        nc.sync.dma_start(
                            out=xlA[a0:a0 + nfa, 0:260].rearrange(
                                "(t p) c -> p t c", p=64),
                            in_=xo[0:64, 0:NW - 1, 0:260])
                        nc.sync.dma_start(
                            out=xlA[a0 + nfa:a0 + 3125, 0:260],
                            in_=xo[0:53, NW - 1, 0:260])
                        nc.sync.dma_start(
                            out=xlB[a0:a0 + nfa, 0:260].rearrange(
                                "(t p) c -> p t c", p=64),
                            in_=xo[64:128, 0:NW - 1, 0:260])
                        nc.sync.dma_start(
                            out=xlB[a0 + nfa:a0 + 3125, 0:260],
                            in_=xo[53:LASTW, NW - 1, 0:260])
                        nc.sync.dma_start(
                            out=adst_d[SHARD * s:SHARD * s + nfull,
                                       :].rearrange("(t p) c -> p t c", p=128),
                            in_=ad[:, 0:NW - 1, :])
                        nc.sync.dma_start(
                            out=adst_d[SHARD * s + nfull:SHARD * (s + 1), :],
                            in_=ad[0:LASTW, NW - 1, :])

                # ---- edge phase: per 128-dst window, software-pipelined ----
                with tc.tile_pool(name="eg", bufs=2) as eg, \
                     tc.tile_pool(name="es", bufs=2) as es, \
                     tc.tile_pool(name="em", bufs=3) as em, \
                     tc.tile_pool(name="er", bufs=4) as er, \
                     tc.tile_pool(name="eo", bufs=2) as eo, \
                     tc.tile_pool(name="agg", bufs=2, space="PSUM") as aggp, \
                     tc.tile_pool(name="adwp", bufs=2, space="PSUM") as adwp, \
                     tc.tile_pool(name="etp", bufs=2, space="PSUM") as etp:
                    grp = {}

                    def emit_loads(w0):
                        gn = min(GW, NW - w0)
                        grp["gn"], grp["w0"] = gn, w0
                        mp = es.tile([128, GW, SW, 128], bf16, tag="mp")
                        nc.sync.dma_start(
                            out=mp[:, 0:gn, :, :].rearrange(
                                "p w s j -> p w (s j)"),
                            in_=Mpure[w0:w0 + gn].rearrange("w p k -> p w k"))
                        mpt = es.tile([128, GW, SW, 128], bf16, tag="mpt")
                        nc.sync.dma_start(
                            out=mpt[:, 0:gn, :, :].rearrange(
                                "p w s j -> p w (s j)"),
                            in_=MpureT[w0:w0 + gn].rearrange("w p k -> p w k"))
                        ia = es.tile([128, GW, SA * 8], mybir.dt.int16,
                                     tag="ia")
                        nc.sync.dma_start(
                            out=ia[:, 0:gn, :],
                            in_=A_idx[w0:w0 + gn].rearrange("w p k -> p w k"))
                        ib = es.tile([128, GW, SB * 8], mybir.dt.int16,
                                     tag="ib")
                        nc.sync.dma_start(
                            out=ib[:, 0:gn, :],
                            in_=B_idx[w0:w0 + gn].rearrange("w p k -> p w k"))
                        # a_dst rows for THIS core's windows: per-core index
                        # param (the program is shared across cores, so a
                        # plain slice would read core 0's rows)
                        awi = es.tile([128, GW], mybir.dt.int32, tag="awi")
                        nc.sync.dma_start(
                            out=awi[:, 0:gn],
                            in_=adst_widx[w0:w0 + gn].rearrange(
                                "w p o -> p (w o)"))
                        aw = es.tile([128, GW, 2], bf16, tag="aw")
                        for wi in range(gn):
                            nc.gpsimd.indirect_dma_start(
                                out=aw[:, wi, :], out_offset=None,
                                in_=adst_d[:],
                                in_offset=bass.IndirectOffsetOnAxis(
                                    ap=awi[:, wi:wi + 1], axis=0))
                        gA = eg.tile([128, GW, SA, ROWC], bf16, tag="gA")
                        gB = eg.tile([128, GW, SB, ROWC], bf16, tag="gB")
                        for wi in range(gn):
                            for (g, xh, it, S_h) in ((gA, xlA, ia, SA),
                                                     (gB, xlB, ib, SB)):
                                s0 = 0
                                while s0 < S_h:   # <=1024 idxs per ucode call
                                    ns = min(8, S_h - s0)
                                    nc.gpsimd.dma_gather(
                                        out_ap=g[:, wi, s0:s0 + ns, :],
                                        in_ap=xh[:],
                                        idxs_ap=it[:, wi, s0 * 8:
                                                   (s0 + ns) * 8],
                                        num_idxs=ns * 128,
                                        num_idxs_reg=ns * 128,
                                        elem_size=ROWC, queue_num=swdge_q())
                                    s0 += ns
                        grp.update(mp=mp, mpt=mpt, aw=aw, gA=gA, gB=gB)

                    def emit_stage1(w):
                        wi = w - grp["w0"]
                        mp, mpt, aw = grp["mp"], grp["mpt"], grp["aw"]
                        gA, gB = grp["gA"], grp["gB"]
                        adw = adwp.tile([128, SW, 2], f32, tag="adw")
                        for si in range(SW):
                            nc.tensor.matmul(
                                adw[:, si, :], lhsT=mpt[:, wi, si, :],
                                rhs=aw[:, wi, :], start=True, stop=True)
                        lg = em.tile([128, SW, 2], f32, tag="lg")
                        nc.vector.tensor_tensor(
                            out=lg[:, 0:SA, :], in0=gA[:, wi, :, 258:260],
                            in1=adw[:, 0:SA, :], op=OP.add)
                        nc.vector.tensor_tensor(
                            out=lg[:, SA:SW, :], in0=gB[:, wi, :, 258:260],
                            in1=adw[:, SA:SW, :], op=OP.add)
                        lr = em.tile([128, SW, 2], f32, tag="lr")
                        nc.vector.scalar_tensor_tensor(
                            out=lr[:], in0=lg[:], scalar=NEG_SLOPE,
                            in1=lg[:], op0=OP.mult, op1=OP.max)
                        ex = em.tile([128, SW, 2], f32, tag="ex")
                        nc.scalar.activation(ex[:], lr[:], AF.Exp)
                        return dict(w=w, wi=wi, ex=ex, mp=grp["mp"],
                                    gA=gA, gB=gB)

                    def emit_stage2(ctx):
                        w, wi, ex = ctx["w"], ctx["wi"], ctx["ex"]
                        mp, gA, gB = ctx["mp"], ctx["gA"], ctx["gB"]
                        nwn = 128 if w < NW - 1 else LASTW
                        pagg = aggp.tile([128, 258], f32, tag="agg")
                        for si in range(SW):
                            g, sih = (gA, si) if si < SA else (gB, si - SA)
                            r = er.tile([128, 258], bf16, tag="r")
                            nc.vector.tensor_scalar(
                                out=r[:, 0:129], in0=g[:, wi, sih, 0:129],
                                scalar1=ex[:, si, 0:1], scalar2=None,
                                op0=OP.mult)
                            nc.scalar.activation(
                                r[:, 129:258], g[:, wi, sih, 129:258],
                                AF.Copy, scale=ex[:, si, 1:2])
                            nc.tensor.matmul(
                                pagg[:], lhsT=mp[:, wi, si, :], rhs=r[:],
                                start=(si == 0), stop=(si == SW - 1))
                        # h_new = 0.5*(msg0*rs0 + msg1*rs1) + bias
                        rs = em.tile([128, 2], f32, tag="rs")
                        nc.vector.tensor_scalar(
                            out=rs[:], in0=pagg[:].rearrange(
                                "p (a b) -> p a b", b=129)[:, :, 128:129],
                            scalar1=1e-16, scalar2=None, op0=OP.add)
                        nc.vector.reciprocal(rs[:], rs[:])
                        nc.vector.tensor_scalar(out=rs[:], in0=rs[:],
                                                scalar1=0.5, scalar2=None,
                                                op0=OP.mult)
                        t0 = em.tile([128, 128], f32, tag="t0")
                        nc.vector.tensor_scalar(out=t0[:], in0=pagg[:, 0:128],
                                                scalar1=rs[:, 0:1],
                                                scalar2=None, op0=OP.mult)
                        t1 = em.tile([128, 128], f32, tag="t1")
                        nc.scalar.activation(t1[:], pagg[:, 129:257],
                                             AF.Copy, scale=rs[:, 1:2])
                        hn = em.tile([128, 128], f32, tag="hn")
                        nc.vector.tensor_tensor(out=hn[:], in0=t0[:],
                                                in1=t1[:], op=OP.add)
                        nc.vector.tensor_tensor(out=hn[:], in0=hn[:],
                                                in1=bias_sb[:], op=OP.add)
                        if conv < NUM_CONVS - 1:
                            pt = etp.tile([128, 128], f32, tag="pt")
                            nc.tensor.transpose(pt[:], hn[:], ident[:])
                            ht = eo.tile([128, 128], bf16, tag="ht")
                            nc.vector.tensor_copy(ht[:, 0:nwn], pt[:, 0:nwn])
                            nc.sync.dma_start(
                                out=h_shT[:, 128 * w:128 * w + nwn],
                                in_=ht[:, 0:nwn])
                        else:
                            ho = eo.tile([128, 128], f32, tag="ho")
                            nc.vector.tensor_copy(ho[:], hn[:])
                            nc.sync.dma_start(
                                out=h_sh[128 * w:128 * w + nwn, :],
                                in_=ho[0:nwn, :])

                    pending = None
                    for w in range(NW):
                        if w % GW == 0:
                            emit_loads(w)
                        ctx = emit_stage1(w)
                        if pending is not None:
                            emit_stage2(pending)
                        pending = ctx
                    emit_stage2(pending)

                if conv < NUM_CONVS - 1:
                    nc.gpsimd.collective_compute(
                        "AllGather", mybir.AluOpType.bypass,
                        ins=[h_shT[:]], outs=[ag_hT[:]],
                        replica_groups=[list(range(NCORES))])
                else:
                    nc.gpsimd.collective_compute(
                        "AllGather", mybir.AluOpType.bypass,
                        ins=[h_sh[:]], outs=[h3_full[:]],
                        replica_groups=[list(range(NCORES))])

            # ---- set2set on this core's 16-graph slice ----
            with tc.tile_pool(name="s2s", bufs=1) as sp, \
                 tc.tile_pool(name="s2w", bufs=2) as swp, \
                 tc.tile_pool(name="s2p", bufs=2, space="PSUM") as s2p, \
                 tc.tile_pool(name="s2g", bufs=1, space="PSUM") as s2g:
                xloc = sp.tile([128, T, 128], f32)
                xidx_sb = sp.tile([128, T], mybir.dt.int32)
                nc.sync.dma_start(
                    out=xidx_sb[:],
                    in_=s2s_xidx.rearrange("t p o -> p (t o)"))
                for t in range(T):
                    nc.gpsimd.indirect_dma_start(
                        out=xloc[:, t, :], out_offset=None, in_=h3_full[:],
                        in_offset=bass.IndirectOffsetOnAxis(
                            ap=xidx_sb[:, t:t + 1], axis=0))
                bl = sp.tile([128, T], f32)
                nc.sync.dma_start(out=bl[:],
                                  in_=s2s_bloc.rearrange("t p o -> p (t o)"))
                brep_sb = sp.tile([16, T, 128], f32)
                nc.sync.dma_start(out=brep_sb[:],
                                  in_=s2s_brep.rearrange("t p d -> p t d"))
                oh = sp.tile([128, T, 16], f32)
                ohT = sp.tile([16, T, 128], f32)
                for t in range(T):
                    nc.vector.tensor_scalar(
                        out=oh[:, t, :], in0=iota16_row[:],
                        scalar1=bl[:, t:t + 1], scalar2=None, op0=OP.is_equal)
                    nc.vector.tensor_scalar(
                        out=ohT[:, t, :], in0=brep_sb[:, t, :],
                        scalar1=iota16_col[:], scalar2=None, op0=OP.is_equal)

                qT = sp.tile([128, 16], f32)
                nc.vector.memset(qT[:], 0.0)
                rT = sp.tile([128, 16], f32)
                nc.vector.memset(rT[:], 0.0)
                cst = sp.tile([16, 128], f32)
                nc.vector.memset(cst[:], 0.0)
                eloc = sp.tile([128, T], f32)

                for step in range(AGGR_STEPS):
                    pg = s2g.tile([16, 512], f32, tag="acc")
                    nc.tensor.matmul(pg[:], lhsT=qT[:], rhs=wia[:],
                                     start=True, stop=False)
                    nc.tensor.matmul(pg[:], lhsT=rT[:], rhs=wib[:],
                                     start=False, stop=False)
                    nc.tensor.matmul(pg[:], lhsT=qT[:], rhs=whh[:],
                                     start=False, stop=True)
                    gt = swp.tile([16, 512], f32, tag="gt")
                    nc.vector.tensor_tensor(out=gt[:], in0=pg[:], in1=bg_sb[:],
                                            op=OP.add)
                    sf = swp.tile([16, 128], f32, tag="sf")
                    nc.scalar.activation(sf[:], gt[:, 128:256], AF.Sigmoid)
                    si_ = swp.tile([16, 128], f32, tag="si")
                    nc.scalar.activation(si_[:], gt[:, 0:128], AF.Sigmoid)
                    tg = swp.tile([16, 128], f32, tag="tg")
                    nc.scalar.activation(tg[:], gt[:, 256:384], AF.Tanh)
                    so = swp.tile([16, 128], f32, tag="so")
                    nc.scalar.activation(so[:], gt[:, 384:512], AF.Sigmoid)
                    c2 = swp.tile([16, 128], f32, tag="c2")
                    nc.vector.tensor_tensor(out=c2[:], in0=sf[:], in1=cst[:],
                                            op=OP.mult)
                    it_ = swp.tile([16, 128], f32, tag="it")
                    nc.vector.tensor_tensor(out=it_[:], in0=si_[:], in1=tg[:],
                                            op=OP.mult)
                    nc.vector.tensor_tensor(out=c2[:], in0=c2[:], in1=it_[:],
                                            op=OP.add)
                    nc.vector.tensor_copy(cst[:], c2[:])
                    tc2 = swp.tile([16, 128], f32, tag="tc2")
                    nc.scalar.activation(tc2[:], c2[:], AF.Tanh)
                    qpad = swp.tile([128, 128], f32, tag="qpad")
                    nc.vector.memset(qpad[:], 0.0)
                    nc.vector.tensor_tensor(out=qpad[0:16, :], in0=so[:],
                                            in1=tc2[:], op=OP.mult)
                    ptq = s2p.tile([128, 128], f32, tag="tp")
                    nc.tensor.transpose(ptq[:], qpad[:], ident[:])
                    nc.vector.tensor_copy(qT[:], ptq[:, 0:16])

                    # e_n = x_n . q[batch_n]
                    for t in range(T):
                        pqx = s2p.tile([128, 128], f32, tag="tp")
                        nc.tensor.matmul(pqx[:], lhsT=ohT[:, t, :],
                                         rhs=qpad[0:16, :], start=True,
                                         stop=True)
                        xq = swp.tile([128, 128], f32, tag="xq")
                        nc.vector.tensor_tensor(out=xq[:], in0=xloc[:, t, :],
                                                in1=pqx[:], op=OP.mult)
                        nc.vector.tensor_reduce(
                            out=eloc[:, t:t + 1], in_=xq[:],
                            axis=mybir.AxisListType.X, op=OP.add)
                    # global (per-core) max for stability
                    mx = swp.tile([128, 1], f32, tag="mx")
                    nc.vector.tensor_reduce(out=mx[:], in_=eloc[:],
                                            axis=mybir.AxisListType.X,
                                            op=OP.max)
                    mpad = swp.tile([128, 128], f32, tag="mpad")
                    nc.vector.memset(mpad[:], -1e30)
                    nc.vector.tensor_copy(mpad[:, 0:1], mx[:])
                    ptm = s2p.tile([128, 128], f32, tag="tp")
                    nc.tensor.transpose(ptm[:], mpad[:], ident[:])
                    msc = swp.tile([1, 1], f32, tag="msc")
                    nc.vector.tensor_reduce(out=msc[:], in_=ptm[0:1, :],
                                            axis=mybir.AxisListType.X,
                                            op=OP.max)
                    pnm = s2p.tile([128, 1], f32, tag="tp")
                    nc.tensor.matmul(pnm[:], lhsT=negones_row[:], rhs=msc[:],
                                     start=True, stop=True)
                    negm = swp.tile([128, 1], f32, tag="negm")
                    nc.vector.tensor_copy(negm[:], pnm[:])

                    pr = s2g.tile([16, 129], f32, tag="acc")
                    for t in range(T):
                        ev = swp.tile([128, 1], f32, tag="ev")
                        nc.scalar.activation(ev[:], eloc[:, t:t + 1], AF.Exp,
                                             bias=negm[:, 0:1])
                        msg = swp.tile([128, 129], f32, tag="msg")
                        nc.scalar.activation(msg[:, 0:128], xloc[:, t, :],
                                             AF.Copy, scale=ev[:, 0:1])
                        nc.vector.tensor_copy(msg[:, 128:129], ev[:])
                        nc.tensor.matmul(pr[:], lhsT=oh[:, t, :], rhs=msg[:],
                                         start=(t == 0), stop=(t == T - 1))
                    rsum = swp.tile([16, 1], f32, tag="rsum")
                    nc.vector.tensor_scalar(out=rsum[:], in0=pr[:, 128:129],
                                            scalar1=1e-16, scalar2=None,
                                            op0=OP.add)
                    nc.vector.reciprocal(rsum[:], rsum[:])
                    rpad = swp.tile([128, 128], f32, tag="rpad")
                    nc.vector.memset(rpad[:], 0.0)
                    nc.vector.tensor_scalar(out=rpad[0:16, :],
                                            in0=pr[:, 0:128],
                                            scalar1=rsum[:, 0:1],
                                            scalar2=None, op0=OP.mult)
                    ptr = s2p.tile([128, 128], f32, tag="tp")
                    nc.tensor.transpose(ptr[:], rpad[:], ident[:])
                    nc.vector.tensor_copy(rT[:], ptr[:, 0:16])

                # MLP head
                pm1 = s2g.tile([16, 128], f32, tag="acc")
                nc.tensor.matmul(pm1[:], lhsT=qT[:], rhs=w1a_sb[:],
                                 start=True, stop=False)
                nc.tensor.matmul(pm1[:], lhsT=rT[:], rhs=w1b_sb[:],
                                 start=False, stop=True)
                hidp = swp.tile([128, 128], f32, tag="hidp")
                nc.vector.memset(hidp[:], 0.0)
                nc.vector.tensor_tensor(out=hidp[0:16, :], in0=pm1[:],
                                        in1=b1_sb[:], op=OP.add)
                nc.scalar.activation(hidp[0:16, :], hidp[0:16, :], AF.Relu)
                pth = s2p.tile([128, 128], f32, tag="tp")
                nc.tensor.transpose(pth[:], hidp[:], ident[:])
                hT_m = swp.tile([128, 16], f32, tag="hTm")
                nc.vector.tensor_copy(hT_m[:], pth[:, 0:16])
                pm2 = s2g.tile([16, 128], f32, tag="acc")
                nc.tensor.matmul(pm2[:], lhsT=hT_m[:], rhs=w2_sb[:],
                                 start=True, stop=True)
                osb = swp.tile([16, 128], f32, tag="osb")
                nc.vector.tensor_tensor(out=osb[:], in0=pm2[:], in1=b2_sb[:],
                                        op=OP.add)
                nc.sync.dma_start(out=out[:], in_=osb[:])

    nc.compile()
    _split_waits(nc)
    return nc


# ---------------------------------------------------------------- entry
def kernel(x, edge_index, edge_attr, batch_index,
           gat_W, gat_att_src, gat_att_dst, gat_bias,
           lstm_Wih, lstm_Whh, lstm_bih, lstm_bhh,
           mlp_W1, mlp_b1, mlp_W2, mlp_b2, _trace=False):
    del edge_attr
    x = np.asarray(x, np.float32)
    edge_index = np.asarray(edge_index)
    batch_index = np.asarray(batch_index)

    cfg, per_core = _host_prep(x, edge_index, batch_index,
                               gat_W, gat_att_src, gat_att_dst)

    Wih = np.asarray(lstm_Wih, np.float32)     # [512, 256]
    Whh = np.asarray(lstm_Whh, np.float32)     # [512, 128]
    WihT = Wih.T.copy()                        # [256, 512]
    bias_gates = (np.asarray(lstm_bih, np.float32)
                  + np.asarray(lstm_bhh, np.float32))
    common = dict(
        h0T=cfg["h0T"], W_eff=cfg["W_eff"],
        bias_rep=np.tile(np.asarray(gat_bias, np.float32)[None, :],
                         (128, 1)),
        WihT_a=WihT[0:128], WihT_b=WihT[128:256],
        WhhT=Whh.T.copy(),
        bg_rep=np.tile(bias_gates[None, :], (16, 1)),
        W1a=np.asarray(mlp_W1, np.float32)[0:128],
        W1b=np.asarray(mlp_W1, np.float32)[128:256],
        W2=np.asarray(mlp_W2, np.float32),
        b1_rep=np.tile(np.asarray(mlp_b1, np.float32)[None, :], (16, 1)),
        b2_rep=np.tile(np.asarray(mlp_b2, np.float32)[None, :], (16, 1)),
    )

    key = (cfg["SA"], cfg["SB"], cfg["T"])
    if _cached.get("key") != key:
        _cached["nc"] = _build(cfg)
        _cached["key"] = key
    nc = _cached["nc"]

    in_maps = []
    for c in range(NCORES):
        m = dict(common)
        m.update(per_core[c])
        m = {k: np.ascontiguousarray(v) for k, v in m.items()}
        in_maps.append(m)

    from concourse.bass_utils import run_bass_kernel_spmd
    res = run_bass_kernel_spmd(nc, in_maps, core_ids=list(range(NCORES)),
                               trace=_trace)
    outp = np.concatenate([res.results[c]["out"] for c in range(NCORES)],
                          axis=0)
    if _trace:
        _cached["last_exec_ns"] = res.exec_time_ns
        _cached["last_res"] = res
    return outp


# revision 27
# speedup vs baseline: 3.9614x; 1.4141x over previous
"""GAT (3 convs) + Set2Set + MLP on 8 Trainium2 NeuronCores — v2.

Sharding: nodes in 8 ranges of 6250; edges (incl self-loops) sorted by dst,
sharded by dst range; per-dst segment softmax is core-local.

v2 design (vs v1): all conv traffic in bf16; one-hot edge->dst masks are
precomputed on host (Mpure [e,j] and MpureT [j,e] per 128-edge slab) and
DMA-loaded instead of built per-slab with slow tensor_scalar ops; the
leakyrelu/exp logit chain is batched per 128-dst window; message aggregation
is one bf16 matmul per slab (lhsT=Mpure, rhs = per-edge rows scaled by
exp(logit) per head: DVE scales head0, Scalar head1); a_dst broadcast to
edges via matmul with MpureT.  Gathered row layout (bf16, 384 cols = 768 B):
[xl0(128) | 1 | xl1(128) | 1 | asrc(2) | pad].  Phase 1 loads whole-shard hT
tiles and writes xl in shard-sized batches.  Set2Set+MLP unchanged (fp32).
"""
import os
import sys

import numpy as np
import ml_dtypes

sys.path.insert(0, "/opt/trn_rl_repo")

BF = ml_dtypes.bfloat16

N, E, F_RAW, D, H, B = 50000, 800000, 9, 128, 2, 128
NUM_CONVS = int(os.environ.get("K_CONVS", "3"))
AGGR_STEPS = int(os.environ.get("K_STEPS", "3"))
NEG_SLOPE = 0.2
NCORES = 8
SHARD = N // NCORES            # 6250
HALF = N // 2                  # 25000
NW = (SHARD + 127) // 128      # 49 windows per core
LASTW = SHARD - (NW - 1) * 128 # 106
ROWC = 384                     # gathered xl row, bf16 cols (768B, %256==0)
GW = 2                         # windows per DMA load-group

_cached = {}


# ---------------------------------------------------------------- patches
def _install_patches():
    import concourse.tile as tile_mod
    from concourse.vector_clock import ScopedClock, VectorClock

    if not getattr(tile_mod.TileContext, "_drain_patched", False):
        def patched(self, tick_clock, wait_clock):
            gc = tick_clock.global_clock
            vals = [gc[p] for p in range(27)]
            for p in [p for p in range(27) if vals[p] > 0]:
                sub = [vals[q] if q == p else 0 for q in range(27)]
                nop = self.nc.sync.nop(nofuse=True, hint="drain_wait_split")
                wait_clock.add_sem_waits(
                    nop.ins, ScopedClock({None: VectorClock(sub)}))
            self.nc.sync.drain()
            self.nc.all_engine_barrier()
            popped = self.nc._tile_sem_poison_stack.pop()
            assert popped is self._sem_poison
            self.nc.clear_and_free_semaphores(
                list(self.sems.allocated().values()))
            self.nc.all_engine_barrier()

        tile_mod.TileContext._drain_and_barrier = patched
        tile_mod.TileContext._drain_patched = True


def _split_waits(nc, max_waits=1):
    """walrus allows at most one sync-wait command per instruction;
    spread extras across injected same-engine NoOps."""
    from concourse import mybir
    n = 0
    for f in nc.m.functions:
        for bb in f.blocks:
            changed, new = False, []
            for ins in bb.instructions:
                si = ins.sync_info
                if si is not None and len(si.on_wait) > max_waits:
                    waits = list(si.on_wait)
                    for i, w in enumerate(waits[max_waits:]):
                        nop = mybir.InstNoOp(
                            name=f"{ins.name}-ws{i}", ins=[], outs=[])
                        nop.engine = ins.engine
                        nop.sync_info = mybir.SyncInfo(
                            on_wait=[w], on_update=[])
                        new.append(nop)
                    ins.sync_info = mybir.SyncInfo(
                        on_wait=waits[:max_waits],
                        on_update=list(si.on_update))
                    changed = True
                    n += 1
                new.append(ins)
            if changed:
                bb.instructions = new
    return n


# ---------------------------------------------------------------- host prep
def _wrap16(flat):
    """dma_gather index layout: idx k at [k%16, k//16], replicated to 128."""
    k = flat.shape[0]
    w = flat.reshape(k // 16, 16).T.astype(np.int16)
    return np.tile(w, (8, 1))


def _pack(src, dst):
    """Permute each core's nodes into windows balancing per-window edge
    counts, and split nodes across the two xl tensors (exactly 64 A-slots
    per full window, 53 in the last) balancing per-window per-half counts.
    Returns pos[n] (global permuted position), ha[n] (0=xlA, 1=xlB)."""
    din = np.zeros(N, np.int64)
    np.add.at(din, dst, 1)
    win_of_node = np.empty(N, np.int64)     # global window id
    for c in range(NCORES):
        nodes = np.arange(SHARD * c, SHARD * (c + 1))
        d = din[nodes]
        order = np.argsort(-d, kind="stable")
        caps = np.full(NW, 128, np.int64)
        caps[-1] = LASTW
        loads = np.zeros(NW, np.int64)
        sizes = np.zeros(NW, np.int64)
        capw = caps / caps.sum()
        for i in order:
            score = (loads + d[i]) / capw
            score[sizes >= caps] = np.inf
            g = int(np.argmin(score))
            win_of_node[nodes[i]] = c * NW + g
            sizes[g] += 1
            loads[g] += d[i]

    GTOT = NCORES * NW
    acap = np.full(GTOT, 64, np.int64)
    acap[NW - 1::NW] = 53
    bcap = np.full(GTOT, 128, np.int64)
    bcap[NW - 1::NW] = LASTW
    bcap -= acap
    eorder = np.argsort(src, kind="stable")
    ew = win_of_node[dst[eorder]]
    estart = np.searchsorted(src[eorder], np.arange(N + 1))
    outdeg = np.diff(estart)
    imb = np.zeros(GTOT, np.float64)
    acnt = np.zeros(GTOT, np.int64)
    bcnt = np.zeros(GTOT, np.int64)
    ha = np.empty(N, np.int8)
    for n in np.argsort(-outdeg, kind="stable"):
        hw = win_of_node[n]
        wl = ew[estart[n]:estart[n + 1]]
        prefer_a = imb[wl].sum() < 0
        if acnt[hw] < acap[hw] and (prefer_a or bcnt[hw] >= bcap[hw]):
            ha[n] = 0
            acnt[hw] += 1
            imb[wl] += 0.5
        else:
            ha[n] = 1
            bcnt[hw] += 1
            imb[wl] -= 0.5

    pos = np.empty(N, np.int64)
    base = np.zeros(GTOT + 1, np.int64)
    np.cumsum(np.where(np.arange(GTOT) % NW == NW - 1, LASTW, 128),
              out=base[1:])
    afill = np.zeros(GTOT, np.int64)
    bfill = acap.copy()
    for n in range(N):
        g = win_of_node[n]
        if ha[n] == 0:
            pos[n] = base[g] + afill[g]
            afill[g] += 1
        else:
            pos[n] = base[g] + bfill[g]
            bfill[g] += 1
    return pos, ha, acap


def _host_prep(x, edge_index, batch_index, gat_W, gat_att_src, gat_att_dst):
    cfg = {}
    src = np.concatenate([edge_index[0], np.arange(N, dtype=np.int64)])
    dst = np.concatenate([edge_index[1], np.arange(N, dtype=np.int64)])
    order = np.argsort(dst, kind="stable")
    src, dst = src[order].astype(np.int32), dst[order].astype(np.int32)
    if os.environ.get("K_SAVE_EDGES"):
        np.savez("/tmp/edges.npz", src=src, dst=dst)

    pos, ha, acap = _pack(src, dst)
    GTOT = NCORES * NW
    base = np.zeros(GTOT + 1, np.int64)
    np.cumsum(np.where(np.arange(GTOT) % NW == NW - 1, LASTW, 128),
              out=base[1:])
    abase = np.zeros(GTOT + 1, np.int64)
    np.cumsum(acap, out=abase[1:])
    bcapg = np.where(np.arange(GTOT) % NW == NW - 1, LASTW, 128) - acap
    bbase = np.zeros(GTOT + 1, np.int64)
    np.cumsum(bcapg, out=bbase[1:])
    g_of = (pos // SHARD) * NW + (pos % SHARD) // 128
    loc = pos - base[g_of]
    row = np.where(ha == 0, abase[g_of] + loc,
                   bbase[g_of] + (loc - acap[g_of]))

    # per (core, window, half): edge lists keyed by PERMUTED dst position
    pdst = pos[dst]
    core_of = pdst // SHARD
    win_of = (pdst % SHARD) // 128
    half_of = ha[src].astype(np.int32)
    key = ((core_of * NW + win_of) * 2 + half_of)
    korder = np.argsort(key, kind="stable")
    src_s, dst_s = src[korder], dst[korder]
    counts = np.bincount(key[korder], minlength=NCORES * NW * 2).reshape(
        NCORES, NW, 2)
    SA = int(np.ceil(counts[:, :, 0].max() / 128))
    SB = int(np.ceil(counts[:, :, 1].max() / 128))
    SW = SA + SB
    cfg["SA"], cfg["SB"], cfg["SW"] = SA, SB, SW

    starts = np.zeros(NCORES * NW * 2 + 1, np.int64)
    np.cumsum(counts.reshape(-1), out=starts[1:])

    # graph boundaries for set2set
    goff = np.searchsorted(batch_index, np.arange(B + 1))
    rows_per_core = np.array(
        [goff[16 * (c + 1)] - goff[16 * c] for c in range(NCORES)])
    T = int(np.ceil(rows_per_core.max() / 128))
    cfg["T"] = T

    # replicated weights: W_eff bf16 [128, 264]
    W = np.asarray(gat_W, np.float32)              # [128, 256]
    asrc_v = np.asarray(gat_att_src, np.float32)   # [2, 128]
    adst_v = np.asarray(gat_att_dst, np.float32)
    w_as = np.stack([W[:, h * D:(h + 1) * D] @ asrc_v[h] for h in range(H)],
                    axis=1)                        # [128, 2]
    w_ad = np.stack([W[:, h * D:(h + 1) * D] @ adst_v[h] for h in range(H)],
                    axis=1)
    W_eff = np.zeros((D, 264), np.float32)
    W_eff[:, 0:128] = W[:, 0:128]
    W_eff[:, 129:257] = W[:, 128:256]
    W_eff[:, 258:260] = w_as
    W_eff[:, 260:262] = w_ad
    cfg["W_eff"] = W_eff.astype(BF)

    xp = np.zeros((N, D), np.float32)
    xp[:, :F_RAW] = x
    invpos = np.empty(N, np.int64)
    invpos[pos] = np.arange(N)
    h0T = np.zeros((NCORES * 128, SHARD), np.float32)
    for s in range(NCORES):
        h0T[128 * s:128 * (s + 1), :] = \
            xp[invpos[SHARD * s:SHARD * (s + 1)]].T
    cfg["h0T"] = h0T.astype(BF)

    per_core = []
    e_rng = np.arange(128)
    for c in range(NCORES):
        A_idx = np.zeros((NW, 128, SA * 8), np.int16)
        B_idx = np.zeros((NW, 128, SB * 8), np.int16)
        Mp = np.zeros((NW, 128, SW * 128), BF)     # [w, e, si*128 + j]
        MpT = np.zeros((NW, 128, SW * 128), BF)    # [w, j, si*128 + e]
        for w in range(NW):
            for hf, (idx_arr, S_h) in enumerate(((A_idx, SA), (B_idx, SB))):
                k = (c * NW + w) * 2 + hf
                lo, hi = starts[k], starts[k + 1]
                cnt = hi - lo
                flat = np.zeros(S_h * 128, np.int64)  # pad -> row 0 (finite)
                flat[:cnt] = row[src_s[lo:hi]]
                idx_arr[w] = _wrap16(flat)
                dl = np.full(S_h * 128, -1, np.int64)
                dl[:cnt] = pos[dst_s[lo:hi]] % SHARD - 128 * w
                base = 0 if hf == 0 else SA
                for s_ in range(S_h):
                    dls = dl[s_ * 128:(s_ + 1) * 128]
                    v = dls >= 0
                    si = base + s_
                    Mp[w, e_rng[v], si * 128 + dls[v]] = 1
                    MpT[w, dls[v], si * 128 + e_rng[v]] = 1

        adst_widx = np.zeros((NW, 128, 1), np.int32)
        for w in range(NW):
            rows = SHARD * c + 128 * w + np.arange(128)
            adst_widx[w, :, 0] = np.minimum(rows, N - 1)

        # set2set slice
        r0, r1 = goff[16 * c], goff[16 * (c + 1)]
        xidx = np.zeros((T, 128, 1), np.int32)
        bloc = np.full((T, 128, 1), -1.0, np.float32)
        rows = np.arange(T * 128)
        glob = np.minimum(r0 + rows, N - 1)
        xidx[:, :, 0] = pos[glob].reshape(T, 128)
        valid = (r0 + rows) < r1
        bl = np.full(T * 128, -1.0, np.float32)
        bl[valid] = (batch_index[glob[valid]] - 16 * c).astype(np.float32)
        bloc[:, :, 0] = bl.reshape(T, 128)
        brep = np.tile(bl.reshape(T, 1, 128), (1, 16, 1)).astype(np.float32)

        per_core.append(dict(
            A_idx=A_idx, B_idx=B_idx, Mpure=Mp, MpureT=MpT,
            adst_widx=adst_widx,
            s2s_xidx=xidx, s2s_bloc=bloc, s2s_brep=brep,
        ))
    return cfg, per_core


# ---------------------------------------------------------------- device build
def _build(cfg):
    import concourse.bacc as bacc
    import concourse.bass as bass
    import concourse.tile as tile
    from concourse import mybir
    from concourse.bass import broadcast_tensor_aps
    from concourse.masks import make_identity

    _install_patches()
    f32 = mybir.dt.float32
    bf16 = mybir.dt.bfloat16
    AF = mybir.ActivationFunctionType
    OP = mybir.AluOpType
    SA, SB, SW, T = cfg["SA"], cfg["SB"], cfg["SW"], cfg["T"]

    nc = bacc.Bacc("TRN2", num_swdge_queues=4)
    P_ = nc.declare_dram_parameter
    h0T = P_("h0T", [NCORES * 128, SHARD], bf16, isOutput=False)
    W_eff = P_("W_eff", [D, 264], bf16, isOutput=False)
    bias_rep = P_("bias_rep", [128, 128], f32, isOutput=False)
    A_idx = P_("A_idx", [NW, 128, SA * 8], mybir.dt.int16, isOutput=False)
    B_idx = P_("B_idx", [NW, 128, SB * 8], mybir.dt.int16, isOutput=False)
    Mpure = P_("Mpure", [NW, 128, SW * 128], bf16, isOutput=False)
    MpureT = P_("MpureT", [NW, 128, SW * 128], bf16, isOutput=False)
    adst_widx = P_("adst_widx", [NW, 128, 1], mybir.dt.int32, isOutput=False)
    s2s_xidx = P_("s2s_xidx", [T, 128, 1], mybir.dt.int32, isOutput=False)
    s2s_bloc = P_("s2s_bloc", [T, 128, 1], f32, isOutput=False)
    s2s_brep = P_("s2s_brep", [T, 16, 128], f32, isOutput=False)
    WihT_a = P_("WihT_a", [128, 512], f32, isOutput=False)
    WihT_b = P_("WihT_b", [128, 512], f32, isOutput=False)
    WhhT = P_("WhhT", [128, 512], f32, isOutput=False)
    bg_rep = P_("bg_rep", [16, 512], f32, isOutput=False)
    W1a = P_("W1a", [128, 128], f32, isOutput=False)
    W1b = P_("W1b", [128, 128], f32, isOutput=False)
    W2 = P_("W2", [128, 128], f32, isOutput=False)
    b1_rep = P_("b1_rep", [16, 128], f32, isOutput=False)
    b2_rep = P_("b2_rep", [16, 128], f32, isOutput=False)
    out = P_("out", [16, 128], f32, isOutput=True)

    if os.environ.get("K_DEBUG"):
        xlA = P_("xlA", [HALF, ROWC], bf16, isOutput=True)
        xlB = P_("xlB", [HALF, ROWC], bf16, isOutput=True)
        adst_d = P_("adst_d", [N, 2], bf16, isOutput=True)
    else:
        xlA = nc.dram_tensor("xlA", [HALF, ROWC], bf16)
        xlB = nc.dram_tensor("xlB", [HALF, ROWC], bf16)
        adst_d = nc.dram_tensor("adst_d", [N, 2], bf16)
    h_shT = nc.dram_tensor("h_shT", [128, SHARD], bf16)
    ag_hT = nc.dram_tensor("ag_hT", [NCORES * 128, SHARD], bf16,
                           addr_space="Shared")
    h_sh = nc.dram_tensor("h_sh", [SHARD, 128], f32)
    h3_full = nc.dram_tensor("h3_full", [N, 128], f32, addr_space="Shared")

    # CoreSim locks each DMASW sem lane to one SWDGE queue (indirect_dma's
    # qPoolDynamic counts as queue 0), so the simulator needs everything on
    # queue 0.  Real HW tolerates mixed queues (v1 ran correct that way),
    # and a single queue serializes gather DMA flow — so rotate on HW.
    _sq = [0]
    _single_q = bool(os.environ.get("K_SINGLE_QUEUE"))

    def swdge_q():
        if _single_q:
            return 0
        v = _sq[0] % 4
        _sq[0] += 1
        return v

    with tile.TileContext(nc) as tc:
        with tc.tile_pool(name="consts", bufs=1) as cp:
            ident = cp.tile([128, 128], f32)
            make_identity(nc, ident[:])
            iota16_row = cp.tile([128, 16], f32)
            nc.gpsimd.iota(iota16_row[:], pattern=[[1, 16]], base=0,
                           channel_multiplier=0,
                           allow_small_or_imprecise_dtypes=True)
            iota16_col = cp.tile([16, 1], f32)
            nc.gpsimd.iota(iota16_col[:], pattern=[[0, 1]], base=0,
                           channel_multiplier=1,
                           allow_small_or_imprecise_dtypes=True)
            negones_row = cp.tile([1, 128], f32)
            nc.vector.memset(negones_row[:], -1.0)
            weff_sb = cp.tile([128, 264], bf16)
            nc.sync.dma_start(out=weff_sb[:], in_=W_eff[:])
            bias_sb = cp.tile([128, 128], f32)
            nc.sync.dma_start(out=bias_sb[:], in_=bias_rep[:])
            wia = cp.tile([128, 512], f32)
            nc.sync.dma_start(out=wia[:], in_=WihT_a[:])
            wib = cp.tile([128, 512], f32)
            nc.sync.dma_start(out=wib[:], in_=WihT_b[:])
            whh = cp.tile([128, 512], f32)
            nc.sync.dma_start(out=whh[:], in_=WhhT[:])
            bg_sb = cp.tile([16, 512], f32)
            nc.sync.dma_start(out=bg_sb[:], in_=bg_rep[:])
            w1a_sb = cp.tile([128, 128], f32)
            nc.sync.dma_start(out=w1a_sb[:], in_=W1a[:])
            w1b_sb = cp.tile([128, 128], f32)
            nc.sync.dma_start(out=w1b_sb[:], in_=W1b[:])
            w2_sb = cp.tile([128, 128], f32)
            nc.sync.dma_start(out=w2_sb[:], in_=W2[:])
            b1_sb = cp.tile([16, 128], f32)
            nc.sync.dma_start(out=b1_sb[:], in_=b1_rep[:])
            b2_sb = cp.tile([16, 128], f32)
            nc.sync.dma_start(out=b2_sb[:], in_=b2_rep[:])

            for conv in range(NUM_CONVS):
                hT_src = h0T if conv == 0 else ag_hT
                # ---- phase 1: xl = h @ W_eff for all N nodes (bf16) ----
                with tc.tile_pool(name="p1h", bufs=2) as p1h, \
                     tc.tile_pool(name="p1w", bufs=2) as p1w, \
                     tc.tile_pool(name="p1p", bufs=4, space="PSUM") as p1p:
                    for s in range(NCORES):
                        hT_t = p1h.tile([128, SHARD], bf16, tag="hT")
                        nc.sync.dma_start(
                            out=hT_t[:],
                            in_=hT_src[128 * s:128 * (s + 1), :])
                        xo = p1w.tile([128, NW, 264], bf16, tag="xo")
                        ad = p1w.tile([128, NW, 2], bf16, tag="ad")
                        for t in range(NW):
                            nwn = 128 if t < NW - 1 else LASTW
                            ps = p1p.tile([128, 262], f32, tag="p1")
                            nc.tensor.matmul(
                                ps[0:nwn, :],
                                lhsT=hT_t[:, 128 * t:128 * t + nwn],
                                rhs=weff_sb[:, 0:262], start=True, stop=True)
                            eng = nc.vector if t % 2 == 0 else nc.scalar
                            if t % 2 == 0:
                                nc.vector.tensor_copy(xo[0:nwn, t, 0:260],
                                                      ps[0:nwn, 0:260])
                            else:
                                nc.scalar.copy(xo[0:nwn, t, 0:260],
                                               ps[0:nwn, 0:260])
                            nc.vector.memset(
                                xo[0:nwn, t, 0:258].rearrange(
                                    "p (a b) -> p a b", b=129)[:, :, 128:129],
                                1.0)
                            nc.vector.tensor_copy(ad[0:nwn, t, :],
                                                  ps[0:nwn, 260:262])
                        # xl rows split across xlA/xlB: window partitions
                        # [0:64) -> xlA, [64:128) -> xlB (last window 53/53);
                        # each shard owns 3125 contiguous rows in each half
                        a0 = 3125 * s
                        nfa = (NW - 1) * 64      # 3072
                        nfull = (NW - 1) * 128   # 6144
                        nc.sync.dma_start(
                            out=xlA[a0:a0 + nfa, 0:260].rearrange(
                                "(t p) c -> p t c", p=64),
                            in_=xo[0:64, 0:NW - 1, 0:260])
                        nc.sync.dma_start(
                            out=xlA[a0 + nfa:a0 + 3125, 0:260],
                            in_=xo[0:53, NW - 1, 0:260])
                        nc.sync.dma_start(
                            out=xlB[a0:a0 + nfa, 0:260].rearrange(
                                "(t p) c -> p t c", p=64),
                            in_=xo[64:128, 0:NW - 1, 0:260])
                        nc.sync.dma_start(
                            out=xlB[a0 + nfa:a0 + 3125, 0:260],
                            in_=xo[53:LASTW, NW - 1, 0:260])
                        nc.sync.dma_start(
                            out=adst_d[SHARD * s:SHARD * s + nfull,
                                       :].rearrange("(t p) c -> p t c", p=128),
                            in_=ad[:, 0:NW - 1, :])
                        nc.sync.dma_start(
                            out=adst_d[SHARD * s + nfull:SHARD * (s + 1), :],
                            in_=ad[0:LASTW, NW - 1, :])

                # ---- edge phase: per 128-dst window, software-pipelined ----
                with tc.tile_pool(name="eg", bufs=2) as eg, \
                     tc.tile_pool(name="es", bufs=2) as es, \
                     tc.tile_pool(name="em", bufs=3) as em, \
                     tc.tile_pool(name="er", bufs=4) as er, \
                     tc.tile_pool(name="eo", bufs=2) as eo, \
                     tc.tile_pool(name="agg", bufs=2, space="PSUM") as aggp, \
                     tc.tile_pool(name="adwp", bufs=2, space="PSUM") as adwp, \
                     tc.tile_pool(name="etp", bufs=2, space="PSUM") as etp:
                    grp = {}

                    def emit_loads(w0):
                        gn = min(GW, NW - w0)
                        grp["gn"], grp["w0"] = gn, w0
                        mp = es.tile([128, GW, SW, 128], bf16, tag="mp")
                        nc.sync.dma_start(
                            out=mp[:, 0:gn, :, :].rearrange(
                                "p w s j -> p w (s j)"),
                            in_=Mpure[w0:w0 + gn].rearrange("w p k -> p w k"))
                        mpt = es.tile([128, GW, SW, 128], bf16, tag="mpt")
                        nc.sync.dma_start(
                            out=mpt[:, 0:gn, :, :].rearrange(
                                "p w s j -> p w (s j)"),
                            in_=MpureT[w0:w0 + gn].rearrange("w p k -> p w k"))
                        ia = es.tile([128, GW, SA * 8], mybir.dt.int16,
                                     tag="ia")
                        nc.sync.dma_start(
                            out=ia[:, 0:gn, :],
                            in_=A_idx[w0:w0 + gn].rearrange("w p k -> p w k"))
                        ib = es.tile([128, GW, SB * 8], mybir.dt.int16,
                                     tag="ib")
                        nc.sync.dma_start(
                            out=ib[:, 0:gn, :],
                            in_=B_idx[w0:w0 + gn].rearrange("w p k -> p w k"))
                        # a_dst rows for THIS core's windows: per-core index
                        # param (the program is shared across cores, so a
                        # plain slice would read core 0's rows)
                        awi = es.tile([128, GW], mybir.dt.int32, tag="awi")
                        nc.sync.dma_start(
                            out=awi[:, 0:gn],
                            in_=adst_widx[w0:w0 + gn].rearrange(
                                "w p o -> p (w o)"))
                        aw = es.tile([128, GW, 2], bf16, tag="aw")
                        for wi in range(gn):
                            nc.gpsimd.indirect_dma_start(
                                out=aw[:, wi, :], out_offset=None,
                                in_=adst_d[:],
                                in_offset=bass.IndirectOffsetOnAxis(
                                    ap=awi[:, wi:wi + 1], axis=0))
                        gA = eg.tile([128, GW, SA, ROWC], bf16, tag="gA")
                        gB = eg.tile([128, GW, SB, ROWC], bf16, tag="gB")
                        for wi in range(gn):
                            for (g, xh, it, S_h) in ((gA, xlA, ia, SA),
                                                     (gB, xlB, ib, SB)):
                                s0 = 0
                                while s0 < S_h:   # <=1024 idxs per ucode call
                                    ns = min(8, S_h - s0)
                                    nc.gpsimd.dma_gather(
                                        out_ap=g[:, wi, s0:s0 + ns, :],
                                        in_ap=xh[:],
                                        idxs_ap=it[:, wi, s0 * 8:
                                                   (s0 + ns) * 8],
                                        num_idxs=ns * 128,
                                        num_idxs_reg=ns * 128,
                                        elem_size=ROWC, queue_num=swdge_q())
                                    s0 += ns
                        grp.update(mp=mp, mpt=mpt, aw=aw, gA=gA, gB=gB)

                    def emit_stage1(w):
                        wi = w - grp["w0"]
                        mp, mpt, aw = grp["mp"], grp["mpt"], grp["aw"]
                        gA, gB = grp["gA"], grp["gB"]
                        adw = adwp.tile([128, SW, 2], f32, tag="adw")
                        for si in range(SW):
                            nc.tensor.matmul(
                                adw[:, si, :], lhsT=mpt[:, wi, si, :],
                                rhs=aw[:, wi, :], start=True, stop=True)
                        lg = em.tile([128, SW, 2], f32, tag="lg")
                        nc.vector.tensor_tensor(
                            out=lg[:, 0:SA, :], in0=gA[:, wi, :, 258:260],
                            in1=adw[:, 0:SA, :], op=OP.add)
                        nc.vector.tensor_tensor(
                            out=lg[:, SA:SW, :], in0=gB[:, wi, :, 258:260],
                            in1=adw[:, SA:SW, :], op=OP.add)
                        lr = em.tile([128, SW, 2], f32, tag="lr")
                        nc.vector.scalar_tensor_tensor(
                            out=lr[:], in0=lg[:], scalar=NEG_SLOPE,
                            in1=lg[:], op0=OP.mult, op1=OP.max)
                        ex = em.tile([128, SW, 2], f32, tag="ex")
                        nc.scalar.activation(ex[:], lr[:], AF.Exp)
                        return dict(w=w, wi=wi, ex=ex, mp=grp["mp"],
                                    gA=gA, gB=gB)

                    def emit_stage2(ctx):
                        w, wi, ex = ctx["w"], ctx["wi"], ctx["ex"]
                        mp, gA, gB = ctx["mp"], ctx["gA"], ctx["gB"]
                        nwn = 128 if w < NW - 1 else LASTW
                        pagg = aggp.tile([128, 258], f32, tag="agg")
                        for si in range(SW):
                            g, sih = (gA, si) if si < SA else (gB, si - SA)
                            r = er.tile([128, 258], bf16, tag="r")
                            # per-edge exp scaling via stride-0 broadcast
                            # tensor_tensor (scalar-AP tensor_scalar is slow)
                            g0 = g[:, wi, sih, 0:129]
                            e0, g0b = broadcast_tensor_aps(
                                ex[:, si, 0:1], g0)
                            nc.vector.tensor_tensor(
                                out=r[:, 0:129], in0=g0b, in1=e0,
                                op=OP.mult)
                            g1 = g[:, wi, sih, 129:258]
                            if si % 2 == 0:
                                e1, g1b = broadcast_tensor_aps(
                                    ex[:, si, 1:2], g1)
                                nc.vector.tensor_tensor(
                                    out=r[:, 129:258], in0=g1b, in1=e1,
                                    op=OP.mult)
                            else:
                                nc.scalar.activation(
                                    r[:, 129:258], g1,
                                    AF.Copy, scale=ex[:, si, 1:2])
                            nc.tensor.matmul(
                                pagg[:], lhsT=mp[:, wi, si, :], rhs=r[:],
                                start=(si == 0), stop=(si == SW - 1))
                        # h_new = 0.5*(msg0*rs0 + msg1*rs1) + bias
                        rs = em.tile([128, 2], f32, tag="rs")
                        nc.vector.tensor_scalar(
                            out=rs[:], in0=pagg[:].rearrange(
                                "p (a b) -> p a b", b=129)[:, :, 128:129],
                            scalar1=1e-16, scalar2=None, op0=OP.add)
                        nc.vector.reciprocal(rs[:], rs[:])
                        nc.vector.tensor_scalar(out=rs[:], in0=rs[:],
                                                scalar1=0.5, scalar2=None,
                                                op0=OP.mult)
                        t0 = em.tile([128, 128], f32, tag="t0")
                        rs0, p0b = broadcast_tensor_aps(rs[:, 0:1],
                                                        pagg[:, 0:128])
                        nc.vector.tensor_tensor(out=t0[:], in0=p0b, in1=rs0,
                                                op=OP.mult)
                        t1 = em.tile([128, 128], f32, tag="t1")
                        nc.scalar.activation(t1[:], pagg[:, 129:257],
                                             AF.Copy, scale=rs[:, 1:2])
                        hn = em.tile([128, 128], f32, tag="hn")
                        nc.vector.tensor_tensor(out=hn[:], in0=t0[:],
                                                in1=t1[:], op=OP.add)
                        nc.vector.tensor_tensor(out=hn[:], in0=hn[:],
                                                in1=bias_sb[:], op=OP.add)
                        if conv < NUM_CONVS - 1:
                            pt = etp.tile([128, 128], f32, tag="pt")
                            nc.tensor.transpose(pt[:], hn[:], ident[:])
                            ht = eo.tile([128, 128], bf16, tag="ht")
                            nc.vector.tensor_copy(ht[:, 0:nwn], pt[:, 0:nwn])
                            nc.sync.dma_start(
                                out=h_shT[:, 128 * w:128 * w + nwn],
                                in_=ht[:, 0:nwn])
                        else:
                            ho = eo.tile([128, 128], f32, tag="ho")
                            nc.vector.tensor_copy(ho[:], hn[:])
                            nc.sync.dma_start(
                                out=h_sh[128 * w:128 * w + nwn, :],
                                in_=ho[0:nwn, :])

                    pending = None
                    for w in range(NW):
                        if w % GW == 0:
                            emit_loads(w)
                        ctx = emit_stage1(w)
                        if pending is not None:
                            emit_stage2(pending)
                        pending = ctx
                    emit_stage2(pending)

                if conv < NUM_CONVS - 1:
                    nc.gpsimd.collective_compute(
                        "AllGather", mybir.AluOpType.bypass,
                        ins=[h_shT[:]], outs=[ag_hT[:]],
                        replica_groups=[list(range(NCORES))])
                else:
                    nc.gpsimd.collective_compute(
                        "AllGather", mybir.AluOpType.bypass,
                        ins=[h_sh[:]], outs=[h3_full[:]],
                        replica_groups=[list(range(NCORES))])

            # ---- set2set on this core's 16-graph slice ----
            with tc.tile_pool(name="s2s", bufs=1) as sp, \
                 tc.tile_pool(name="s2w", bufs=2) as swp, \
                 tc.tile_pool(name="s2p", bufs=2, space="PSUM") as s2p, \
                 tc.tile_pool(name="s2g", bufs=1, space="PSUM") as s2g:
                xloc = sp.tile([128, T, 128], f32)
                xidx_sb = sp.tile([128, T], mybir.dt.int32)
                nc.sync.dma_start(
                    out=xidx_sb[:],
                    in_=s2s_xidx.rearrange("t p o -> p (t o)"))
                for t in range(T):
                    nc.gpsimd.indirect_dma_start(
                        out=xloc[:, t, :], out_offset=None, in_=h3_full[:],
                        in_offset=bass.IndirectOffsetOnAxis(
                            ap=xidx_sb[:, t:t + 1], axis=0))
                bl = sp.tile([128, T], f32)
                nc.sync.dma_start(out=bl[:],
                                  in_=s2s_bloc.rearrange("t p o -> p (t o)"))
                brep_sb = sp.tile([16, T, 128], f32)
                nc.sync.dma_start(out=brep_sb[:],
                                  in_=s2s_brep.rearrange("t p d -> p t d"))
                oh = sp.tile([128, T, 16], f32)
                ohT = sp.tile([16, T, 128], f32)
                for t in range(T):
                    nc.vector.tensor_scalar(
                        out=oh[:, t, :], in0=iota16_row[:],
                        scalar1=bl[:, t:t + 1], scalar2=None, op0=OP.is_equal)
                    nc.vector.tensor_scalar(
                        out=ohT[:, t, :], in0=brep_sb[:, t, :],
                        scalar1=iota16_col[:], scalar2=None, op0=OP.is_equal)

                qT = sp.tile([128, 16], f32)
                nc.vector.memset(qT[:], 0.0)
                rT = sp.tile([128, 16], f32)
                nc.vector.memset(rT[:], 0.0)
                cst = sp.tile([16, 128], f32)
                nc.vector.memset(cst[:], 0.0)
                eloc = sp.tile([128, T], f32)

                for step in range(AGGR_STEPS):
                    pg = s2g.tile([16, 512], f32, tag="acc")
                    nc.tensor.matmul(pg[:], lhsT=qT[:], rhs=wia[:],
                                     start=True, stop=False)
                    nc.tensor.matmul(pg[:], lhsT=rT[:], rhs=wib[:],
                                     start=False, stop=False)
                    nc.tensor.matmul(pg[:], lhsT=qT[:], rhs=whh[:],
                                     start=False, stop=True)
                    gt = swp.tile([16, 512], f32, tag="gt")
                    nc.vector.tensor_tensor(out=gt[:], in0=pg[:], in1=bg_sb[:],
                                            op=OP.add)
                    sf = swp.tile([16, 128], f32, tag="sf")
                    nc.scalar.activation(sf[:], gt[:, 128:256], AF.Sigmoid)
                    si_ = swp.tile([16, 128], f32, tag="si")
                    nc.scalar.activation(si_[:], gt[:, 0:128], AF.Sigmoid)
                    tg = swp.tile([16, 128], f32, tag="tg")
                    nc.scalar.activation(tg[:], gt[:, 256:384], AF.Tanh)
                    so = swp.tile([16, 128], f32, tag="so")
                    nc.scalar.activation(so[:], gt[:, 384:512], AF.Sigmoid)
                    c2 = swp.tile([16, 128], f32, tag="c2")
                    nc.vector.tensor_tensor(out=c2[:], in0=sf[:], in1=cst[:],
                                            op=OP.mult)
                    it_ = swp.tile([16, 128], f32, tag="it")
                    nc.vector.tensor_tensor(out=it_[:], in0=si_[:], in1=tg[:],
                                            op=OP.mult)
                    nc.vector.tensor_tensor(out=c2[:], in0=c2[:], in1=it_[:],
                                            op=OP.add)
                    nc.vector.tensor_copy(cst[:], c2[:])
                    tc2 = swp.tile([16, 128], f32, tag="tc2")
                    nc.scalar.activation(tc2[:], c2[:], AF.Tanh)
                    qpad = swp.tile([128, 128], f32, tag="qpad")
                    nc.vector.memset(qpad[:], 0.0)
                    nc.vector.tensor_tensor(out=qpad[0:16, :], in0=so[:],
                                            in1=tc2[:], op=OP.mult)
                    ptq = s2p.tile([128, 128], f32, tag="tp")
                    nc.tensor.transpose(ptq[:], qpad[:], ident[:])
                    nc.vector.tensor_copy(qT[:], ptq[:, 0:16])

                    # e_n = x_n . q[batch_n]
                    for t in range(T):
                        pqx = s2p.tile([128, 128], f32, tag="tp")
                        nc.tensor.matmul(pqx[:], lhsT=ohT[:, t, :],
                                         rhs=qpad[0:16, :], start=True,
                                         stop=True)
                        xq = swp.tile([128, 128], f32, tag="xq")
                        nc.vector.tensor_tensor(out=xq[:], in0=xloc[:, t, :],
                                                in1=pqx[:], op=OP.mult)
                        nc.vector.tensor_reduce(
                            out=eloc[:, t:t + 1], in_=xq[:],
                            axis=mybir.AxisListType.X, op=OP.add)
                    # global (per-core) max for stability
                    mx = swp.tile([128, 1], f32, tag="mx")
                    nc.vector.tensor_reduce(out=mx[:], in_=eloc[:],
                                            axis=mybir.AxisListType.X,
                                            op=OP.max)
                    mpad = swp.tile([128, 128], f32, tag="mpad")
                    nc.vector.memset(mpad[:], -1e30)
                    nc.vector.tensor_copy(mpad[:, 0:1], mx[:])
                    ptm = s2p.tile([128, 128], f32, tag="tp")
                    nc.tensor.transpose(ptm[:], mpad[:], ident[:])
                    msc = swp.tile([1, 1], f32, tag="msc")
                    nc.vector.tensor_reduce(out=msc[:], in_=ptm[0:1, :],
                                            axis=mybir.AxisListType.X,
                                            op=OP.max)
                    pnm = s2p.tile([128, 1], f32, tag="tp")
                    nc.tensor.matmul(pnm[:], lhsT=negones_row[:], rhs=msc[:],
                                     start=True, stop=True)
                    negm = swp.tile([128, 1], f32, tag="negm")
                    nc.vector.tensor_copy(negm[:], pnm[:])

                    pr = s2g.tile([16, 129], f32, tag="acc")
                    for t in range(T):
                        ev = swp.tile([128, 1], f32, tag="ev")
                        nc.scalar.activation(ev[:], eloc[:, t:t + 1], AF.Exp,
                                             bias=negm[:, 0:1])
                        msg = swp.tile([128, 129], f32, tag="msg")
                        nc.scalar.activation(msg[:, 0:128], xloc[:, t, :],
                                             AF.Copy, scale=ev[:, 0:1])
                        nc.vector.tensor_copy(msg[:, 128:129], ev[:])
                        nc.tensor.matmul(pr[:], lhsT=oh[:, t, :], rhs=msg[:],
                                         start=(t == 0), stop=(t == T - 1))
                    rsum = swp.tile([16, 1], f32, tag="rsum")
                    nc.vector.tensor_scalar(out=rsum[:], in0=pr[:, 128:129],
                                            scalar1=1e-16, scalar2=None,
                                            op0=OP.add)
                    nc.vector.reciprocal(rsum[:], rsum[:])
                    rpad = swp.tile([128, 128], f32, tag="rpad")
                    nc.vector.memset(rpad[:], 0.0)
                    nc.vector.tensor_scalar(out=rpad[0:16, :],
                                            in0=pr[:, 0:128],
                                            scalar1=rsum[:, 0:1],
                                            scalar2=None, op0=OP.mult)
                    ptr = s2p.tile([128, 128], f32, tag="tp")
                    nc.tensor.transpose(ptr[:], rpad[:], ident[:])
                    nc.vector.tensor_copy(rT[:], ptr[:, 0:16])

                # MLP head
                pm1 = s2g.tile([16, 128], f32, tag="acc")
                nc.tensor.matmul(pm1[:], lhsT=qT[:], rhs=w1a_sb[:],
                                 start=True, stop=False)
                nc.tensor.matmul(pm1[:], lhsT=rT[:], rhs=w1b_sb[:],
                                 start=False, stop=True)
                hidp = swp.tile([128, 128], f32, tag="hidp")
                nc.vector.memset(hidp[:], 0.0)
                nc.vector.tensor_tensor(out=hidp[0:16, :], in0=pm1[:],
                                        in1=b1_sb[:], op=OP.add)
                nc.scalar.activation(hidp[0:16, :], hidp[0:16, :], AF.Relu)
                pth = s2p.tile([128, 128], f32, tag="tp")
                nc.tensor.transpose(pth[:], hidp[:], ident[:])
                hT_m = swp.tile([128, 16], f32, tag="hTm")
                nc.vector.tensor_copy(hT_m[:], pth[:, 0:16])
                pm2 = s2g.tile([16, 128], f32, tag="acc")
                nc.tensor.matmul(pm2[:], lhsT=hT_m[:], rhs=w2_sb[:],
                                 start=True, stop=True)
                osb = swp.tile([16, 128], f32, tag="osb")
                nc.vector.tensor_tensor(out=osb[:], in0=pm2[:], in1=b2_sb[:],
                                        op=OP.add)
                nc.sync.dma_start(out=out[:], in_=osb[:])

    nc.compile()
    _split_waits(nc)
    return nc


# ---------------------------------------------------------------- entry
def kernel(x, edge_index, edge_attr, batch_index,
           gat_W, gat_att_src, gat_att_dst, gat_bias,
           lstm_Wih, lstm_Whh, lstm_bih, lstm_bhh,
           mlp_W1, mlp_b1, mlp_W2, mlp_b2, _trace=False):
    del edge_attr
    x = np.asarray(x, np.float32)
    edge_index = np.asarray(edge_index)
    batch_index = np.asarray(batch_index)

    cfg, per_core = _host_prep(x, edge_index, batch_index,
                               gat_W, gat_att_src, gat_att_dst)

    Wih = np.asarray(lstm_Wih, np.float32)     # [512, 256]
    Whh = np.asarray(lstm_Whh, np.float32)     # [512, 128]
    WihT = Wih.T.copy()                        # [256, 512]
    bias_gates = (np.asarray(lstm_bih, np.float32)
                  + np.asarray(lstm_bhh, np.float32))
    common = dict(
        h0T=cfg["h0T"], W_eff=cfg["W_eff"],
        bias_rep=np.tile(np.asarray(gat_bias, np.float32)[None, :],
                         (128, 1)),
        WihT_a=WihT[0:128], WihT_b=WihT[128:256],
        WhhT=Whh.T.copy(),
        bg_rep=np.tile(bias_gates[None, :], (16, 1)),
        W1a=np.asarray(mlp_W1, np.float32)[0:128],
        W1b=np.asarray(mlp_W1, np.float32)[128:256],
        W2=np.asarray(mlp_W2, np.float32),
        b1_rep=np.tile(np.asarray(mlp_b1, np.float32)[None, :], (16, 1)),
        b2_rep=np.tile(np.asarray(mlp_b2, np.float32)[None, :], (16, 1)),
    )

    key = (cfg["SA"], cfg["SB"], cfg["T"])
    if _cached.get("key") != key:
        _cached["nc"] = _build(cfg)
        _cached["key"] = key
    nc = _cached["nc"]

    in_maps = []
    for c in range(NCORES):
        m = dict(common)
        m.update(per_core[c])
        m = {k: np.ascontiguousarray(v) for k, v in m.items()}
        in_maps.append(m)

    from concourse.bass_utils import run_bass_kernel_spmd
    res = run_bass_kernel_spmd(nc, in_maps, core_ids=list(range(NCORES)),
                               trace=_trace)
    outp = np.concatenate([res.results[c]["out"] for c in range(NCORES)],
                          axis=0)
    if _trace:
        _cached["last_exec_ns"] = res.exec_time_ns
        _cached["last_res"] = res
    return outp
